# revision 1
# baseline (speedup 1.0000x reference)
"""DLSMN scatter-memory + cache self-attention kernel for Trainium2.

Data-parallel over batch: batch b runs on NeuronCore b (8 cores), no
collectives.  Inside one core (one batch):

  phase A: per 128-token tile of y: PE-transpose y -> yT chunks, fused
           matmuls  [W_write | (W_slot,W_gate)]  (fp32r), gumbel-softmax
           routing via exp(logits*gamma - ln(-ln(u+eps)+eps)) (single
           Ln/Exp ACT table set), weighted-scatter matmul with a leading
           ones column in the rhs so the write-mass comes out of the same
           accumulation for free.
  phase B: slot update  upd = (1-g)*DECAY*old + g*updates/(mass+eps).
  phase C: PE-transpose cache2 -> cache2T (bf16).
  phase D: q/k/v projections in bf16 (qT,kT transposed layout; v natural).
  phase E: attention computed transposed: attT[m,n] tiles; softmax has no
           max-subtraction (logits are provably tiny); denominators via
           col-tiled ones-matmuls (4 heads concurrent in the PE array);
           ao^T accumulated in PSUM; normalization by exp(-ln(den)).
  phase F: output projection + residual + layernorm (fused DVE
           scalar_tensor_tensor with accum_out row sums).
"""

import numpy as np

import concourse.bacc as bacc
import concourse.mybir as mybir
import concourse.tile as tile
from concourse.bass_utils import run_bass_kernel_spmd
from concourse.masks import make_identity

F32 = mybir.dt.float32
F32R = mybir.dt.float32r
BF16 = mybir.dt.bfloat16
AF = mybir.ActivationFunctionType
ALU = mybir.AluOpType

B = 8
S = 2048
D = 1024
DC = 512
K = 256
L = 8
H = 4
HD = 128
N = L * K
LAYER_IDX = 3
DECAY = 0.9
EPS = 1e-6
ST = S // 128  # 16 token tiles
NT = N // 128  # 16 slot tiles
DCH = D // 128  # 8 d_model chunks
CL = 256  # attention n-chunk length
NCH = N // CL  # 8 attention chunks
ATT_SCALE = float(1.0 / np.sqrt(np.float32(HD)))

_INPUT_SPECS = {
    "y": (S, D), "cache": (N, DC), "gumbel_u": (S, K),
    "W_gate": (D, 1), "b_gate": (1,), "W_slot": (D, K), "b_slot": (K,),
    "gamma": (1,), "W_write": (D, DC), "b_write": (DC,),
    "Wq": (DC, DC), "bq": (DC,), "Wk": (DC, DC), "bk": (DC,),
    "Wv": (DC, DC), "bv": (DC,), "Wo": (DC, DC), "bo": (DC,),
    "ln_g": (DC,), "ln_b": (DC,),
}


def _r(ap):
    return ap.bitcast(F32R)


def _build():
    nc = bacc.Bacc("TRN2", target_bir_lowering=False, debug=False, num_devices=B)

    a = {
        name: nc.dram_tensor(name, list(shape), F32, kind="ExternalInput").ap()
        for name, shape in _INPUT_SPECS.items()
    }
    out_dram = nc.dram_tensor("out", [N, DC], F32, kind="ExternalOutput").ap()

    y3 = a["y"].rearrange("(t p) d -> p t d", p=128)
    gum3 = a["gumbel_u"].rearrange("(t p) k -> p t k", p=128)
    cache3 = a["cache"].rearrange("(t p) d -> p t d", p=128)
    out3 = out_dram.rearrange("(t p) d -> p t d", p=128)

    with tile.TileContext(nc) as tc:
        with (
            tc.tile_pool(name="const", bufs=1) as const,
            tc.tile_pool(name="cachep", bufs=1) as cachep,
        ):
            ident = const.tile([128, 128], F32)
            make_identity(nc, ident)
            ones_row_f = const.tile([1, DC], F32)
            nc.vector.memset(ones_row_f, 1.0)
            ones_col2_f = const.tile([128, 2], F32)
            nc.vector.memset(ones_col2_f, 1.0)
            ones_row = const.tile([1, DC], F32R)
            nc.vector.tensor_copy(out=ones_row, in_=ones_row_f)
            ones_row_bf = const.tile([1, DC], BF16)
            nc.vector.memset(ones_row_bf, 1.0)
            ones_col_bf = const.tile([128, 1], BF16)
            nc.vector.memset(ones_col_bf, 1.0)
            eps8_t = const.tile([128, 1], F32)
            nc.vector.memset(eps8_t, 1e-8)
            eps5_t = const.tile([128, 1], F32)
            nc.vector.memset(eps5_t, 1e-5)
            gamma_t = const.tile([128, 1], F32)
            nc.sync.dma_start(out=gamma_t, in_=a["gamma"].unsqueeze(0).to_broadcast([128, 1]))
            lng_bc = const.tile([128, DC], F32)
            nc.sync.dma_start(out=lng_bc, in_=a["ln_g"].unsqueeze(0).to_broadcast([128, DC]))
            lnb_bc = const.tile([128, DC], F32)
            nc.sync.dma_start(out=lnb_bc, in_=a["ln_b"].unsqueeze(0).to_broadcast([128, DC]))
            bwr_row = const.tile([1, DC], F32R)
            nc.gpsimd.dma_start(out=bwr_row, in_=a["b_write"].unsqueeze(0))
            bsg_row = const.tile([1, K + 2], F32R)
            nc.gpsimd.dma_start(out=bsg_row[:, 0:K], in_=a["b_slot"].unsqueeze(0))
            nc.gpsimd.dma_start(out=bsg_row[:, K:K + 1], in_=a["b_gate"].unsqueeze(0))
            nc.gpsimd.dma_start(out=bsg_row[:, K + 1:K + 2], in_=a["b_gate"].unsqueeze(0))
            # bf16 bias rows for the attention-side projections
            bqr = const.tile([1, DC], BF16)
            nc.gpsimd.dma_start(out=bqr, in_=a["bq"].unsqueeze(0))
            bkr = const.tile([1, DC], BF16)
            nc.gpsimd.dma_start(out=bkr, in_=a["bk"].unsqueeze(0))
            bvr = const.tile([1, DC], BF16)
            nc.gpsimd.dma_start(out=bvr, in_=a["bv"].unsqueeze(0))
            bor = const.tile([1, DC], BF16)
            nc.gpsimd.dma_start(out=bor, in_=a["bo"].unsqueeze(0))

            cache_sb = cachep.tile([128, NT, DC], F32)

            # ---------------- phase A + B: selection & scatter write ------
            with (
                tc.tile_pool(name="wA", bufs=1) as wA,
                tc.tile_pool(name="pA", bufs=2) as pA,
                tc.tile_pool(name="pAs", bufs=3) as pAs,
                tc.tile_pool(name="psU", bufs=1, space="PSUM") as psU,
                tc.tile_pool(name="psA", bufs=1, space="PSUM") as psA,
                tc.tile_pool(name="psT", bufs=2, space="PSUM") as psT,
            ):
                wwr = wA.tile([128, DCH, DC], F32R)
                wsg = wA.tile([128, DCH, K + 2], F32R)

                # gumbel pre-pass: all Ln ops batched (one ACT table residency)
                lnz_all = wA.tile([128, ST, K], F32)
                for i in range(ST):
                    gum = pA.tile([128, K], F32, tag="gum")
                    nc.sync.dma_start(out=gum, in_=gum3[:, i, :])
                    lnu = pAs.tile([128, K], F32, tag="lnu")
                    nc.scalar.activation(lnu, gum, AF.Ln, bias=eps8_t)
                    nc.scalar.activation(lnz_all[:, i, :], lnu, AF.Ln, bias=eps8_t,
                                         scale=-1.0)

                # persistent scatter accumulators: [ones|wv] x w  ->  [mass | updates]
                ps_ua = [psU.tile([128, K + 2], F32, name=f"ua{kc}", tag=f"ua{kc}")
                         for kc in range(2)]
                ps_ub = [psU.tile([128, K], F32, name=f"ub{kc}", tag=f"ub{kc}")
                         for kc in range(2)]

                pending = []

                def flush_updates():
                    while pending:
                        j, w_j, wv_j = pending.pop(0)
                        for kc in range(2):
                            lhs = w_j[:, kc * 128:(kc + 1) * 128]
                            nc.tensor.matmul(ps_ua[kc], lhs, wv_j[:, 0:K + 2],
                                             start=(j == 0), stop=(j == ST - 1))
                            nc.tensor.matmul(ps_ub[kc], lhs, wv_j[:, K + 2:DC + 2],
                                             start=(j == 0), stop=(j == ST - 1))

                for i in range(ST):
                    y_t = pA.tile([128, D], F32, tag="y")
                    nc.sync.dma_start(out=y_t, in_=y3[:, i, :])
                    if i == 0:
                        wwr3 = a["W_write"].rearrange("(c p) d -> p c d", p=128)
                        wsl3 = a["W_slot"].rearrange("(c p) k -> p c k", p=128)
                        for c in range(DCH):
                            nc.gpsimd.dma_start(out=wwr[:, c, :], in_=wwr3[:, c, :])
                            nc.gpsimd.dma_start(out=wsg[:, c, 0:K], in_=wsl3[:, c, :])
                        nc.gpsimd.dma_start(out=wsg[:, :, K:K + 1], in_=a["W_gate"].rearrange("(c p) o -> p c o", p=128))
                        nc.gpsimd.dma_start(out=wsg[:, :, K + 1:K + 2], in_=a["W_gate"].rearrange("(c p) o -> p c o", p=128))
                    if i == 1:
                        nc.sync.dma_start(out=cache_sb, in_=cache3)

                    # transpose y tile -> yT (8 chunks of [128d, 128s])
                    yT = pA.tile([128, D], F32R, tag="yT")
                    for g in range(2):
                        tr = psT.tile([128, 512], F32, tag="tr")
                        for cc in range(4):
                            c = 4 * g + cc
                            nc.tensor.transpose(
                                tr[:, cc * 128:(cc + 1) * 128],
                                y_t[:, c * 128:(c + 1) * 128],
                                ident,
                            )
                        nc.vector.tensor_copy(out=yT[:, g * 512:(g + 1) * 512], in_=tr)
                    flush_updates()

                    # fused write_vals / (logits, gate) matmuls
                    ps_wv = psA.tile([128, DC], F32, tag="wv")
                    for c in range(DCH):
                        nc.tensor.matmul(
                            ps_wv, yT[:, c * 128:(c + 1) * 128], wwr[:, c, :],
                            start=(c == 0), stop=False,
                        )
                    nc.tensor.matmul(ps_wv, ones_row[:, 0:128], bwr_row,
                                     start=False, stop=True)
                    ps_lg = psA.tile([128, K + 2], F32, tag="lg")
                    for c in range(DCH):
                        nc.tensor.matmul(
                            ps_lg, yT[:, c * 128:(c + 1) * 128], wsg[:, c, :],
                            start=(c == 0), stop=False,
                        )
                    nc.tensor.matmul(ps_lg, ones_row[:, 0:128], bsg_row,
                                     start=False, stop=True)

                    # t = gamma*logits - lnz   (lnz precomputed in the pre-pass)
                    t_sb = pAs.tile([128, K], F32, tag="tsb")
                    nc.vector.scalar_tensor_tensor(
                        out=t_sb, in0=ps_lg[:, 0:K], scalar=gamma_t, in1=lnz_all[:, i, :],
                        op0=ALU.mult, op1=ALU.subtract,
                    )

                    # scores = sigmoid(gate) = 1/(1+exp(-gate))
                    sc_e = pAs.tile([128, 1], F32, tag="sce")
                    nc.scalar.activation(sc_e, ps_lg[:, K:K + 1], AF.Exp, scale=-1.0)
                    sc1 = pAs.tile([128, 1], F32, tag="sc1")
                    nc.vector.tensor_scalar_add(sc1, sc_e, 1.0)
                    scores = pAs.tile([128, 1], F32, tag="scores")
                    nc.vector.reciprocal(scores, sc1)

                    # p_unnorm = exp(t), row-sum fused; w = p_unnorm*(scores/rowsum)
                    p_un = pAs.tile([128, K], F32, tag="pun")
                    rs = pAs.tile([128, 1], F32, tag="rs")
                    nc.scalar.activation(p_un, t_sb, AF.Exp, accum_out=rs)
                    rrs = pAs.tile([128, 1], F32, tag="rrs")
                    nc.vector.reciprocal(rrs, rs)
                    s2 = pAs.tile([128, 1], F32, tag="s2")
                    nc.vector.tensor_tensor(s2, scores, rrs, ALU.mult)
                    w_sb = pAs.tile([128, K], F32R, tag="wsb")
                    nc.vector.tensor_scalar_mul(w_sb, p_un, s2)

                    # wv_sb = [ones | write_vals]
                    wv_sb = pAs.tile([128, DC + 2], F32R, tag="wvsb")
                    nc.vector.tensor_copy(out=wv_sb[:, 0:2], in_=ones_col2_f)
                    nc.vector.tensor_copy(out=wv_sb[:, 2:DC + 2], in_=ps_wv)
                    pending.append((i, w_sb, wv_sb))

                flush_updates()

                # ------- phase B: slot update, overwrite cache rows -------
                base_t = LAYER_IDX * K // 128  # n-tile 6
                for kc in range(2):
                    mass = pAs.tile([128, 1], F32, tag="mass")
                    nc.vector.tensor_copy(out=mass, in_=ps_ua[kc][:, 0:1])
                    m1 = pAs.tile([128, 1], F32, tag="m1")
                    nc.vector.tensor_scalar_add(m1, mass, EPS)
                    rm = pAs.tile([128, 1], F32, tag="rm")
                    nc.vector.reciprocal(rm, m1)
                    m2 = pAs.tile([128, 1], F32, tag="m2")
                    nc.vector.tensor_scalar_add(m2, mass, 1.0)
                    rg = pAs.tile([128, 1], F32, tag="rg")
                    nc.vector.reciprocal(rg, m2)
                    g_t = pAs.tile([128, 1], F32, tag="gt")
                    nc.vector.tensor_tensor(g_t, mass, rg, ALU.mult)
                    co = pAs.tile([128, 1], F32, tag="co")
                    nc.vector.tensor_scalar(co, g_t, -DECAY, DECAY, ALU.mult, ALU.add)
                    cn = pAs.tile([128, 1], F32, tag="cn")
                    nc.vector.tensor_tensor(cn, g_t, rm, ALU.mult)

                    told = pAs.tile([128, DC], F32, tag="told")
                    nc.vector.tensor_scalar_mul(told, cache_sb[:, base_t + kc, :], co)
                    nc.vector.scalar_tensor_tensor(
                        out=cache_sb[:, base_t + kc, 0:K],
                        in0=ps_ua[kc][:, 2:K + 2], scalar=cn, in1=told[:, 0:K],
                        op0=ALU.mult, op1=ALU.add,
                    )
                    nc.vector.scalar_tensor_tensor(
                        out=cache_sb[:, base_t + kc, K:DC],
                        in0=ps_ub[kc], scalar=cn, in1=told[:, K:DC],
                        op0=ALU.mult, op1=ALU.add,
                    )

            # ---------------- phases C-F ----------------------------------
            with (
                tc.tile_pool(name="woP", bufs=1) as woP,
                tc.tile_pool(name="aoP", bufs=1) as aoP,
            ):
                wo_sb = woP.tile([128, H, DC], BF16)
                nc.gpsimd.dma_start(out=wo_sb, in_=a["Wo"].rearrange("(c p) d -> p c d", p=128))
                aoT = aoP.tile([128, H, N], BF16)

                with (
                    tc.tile_pool(name="c2tP", bufs=1) as c2tP,
                    tc.tile_pool(name="wqkvP", bufs=1) as wqkvP,
                    tc.tile_pool(name="qkvP", bufs=1) as qkvP,
                ):
                    # ------- phase C: cache2 -> cache2T (bf16) -----------
                    c2t = c2tP.tile([128, 4, N], BF16)
                    with tc.tile_pool(name="psC", bufs=2, space="PSUM") as psC:
                        for j in range(4):
                            for tg in range(4):
                                ps = psC.tile([128, 512], F32, tag="ctr")
                                for tt in range(4):
                                    t = tg * 4 + tt
                                    nc.tensor.transpose(
                                        ps[:, tt * 128:(tt + 1) * 128],
                                        cache_sb[:, t, j * 128:(j + 1) * 128],
                                        ident,
                                    )
                                nc.scalar.copy(
                                    out=c2t[:, j, tg * 512:(tg + 1) * 512], in_=ps)

                    # ------- phase D: q/k/v projections (bf16) -----------
                    wq_sb = wqkvP.tile([128, 4, DC], BF16)
                    nc.gpsimd.dma_start(out=wq_sb, in_=a["Wq"].rearrange("(c p) d -> p c d", p=128))
                    wk_sb = wqkvP.tile([128, 4, DC], BF16)
                    nc.gpsimd.dma_start(out=wk_sb, in_=a["Wk"].rearrange("(c p) d -> p c d", p=128))
                    wv_w = wqkvP.tile([128, 4, DC], BF16)
                    nc.gpsimd.dma_start(out=wv_w, in_=a["Wv"].rearrange("(c p) d -> p c d", p=128))

                    qT = qkvP.tile([128, H, N], BF16)
                    kT = qkvP.tile([128, H, N], BF16)
                    v_sb = qkvP.tile([128, NT, DC], BF16)

                    with tc.tile_pool(name="psD", bufs=3, space="PSUM") as psD:
                        for dst, w_t, b_t in ((qT, wq_sb, bqr), (kT, wk_sb, bkr)):
                            for h in range(H):
                                for c in range(4):
                                    ps = psD.tile([128, 512], F32, tag="qk")
                                    for j in range(4):
                                        nc.tensor.matmul(
                                            ps, w_t[:, j, h * 128:(h + 1) * 128],
                                            c2t[:, j, c * 512:(c + 1) * 512],
                                            start=(j == 0), stop=False,
                                        )
                                    nc.tensor.matmul(
                                        ps, b_t[:, h * 128:(h + 1) * 128],
                                        ones_row_bf[:, 0:512], start=False, stop=True,
                                    )
                                    nc.scalar.copy(
                                        out=dst[:, h, c * 512:(c + 1) * 512], in_=ps)
                        for m in range(NT):
                            ps = psD.tile([128, DC], F32, tag="v")
                            for j in range(4):
                                nc.tensor.matmul(
                                    ps, c2t[:, j, m * 128:(m + 1) * 128], wv_w[:, j, :],
                                    start=(j == 0), stop=False,
                                )
                            nc.tensor.matmul(ps, ones_row_bf[:, 0:128], bvr,
                                             start=False, stop=True)
                            nc.scalar.copy(out=v_sb[:, m, :], in_=ps)

                    # ------- phase E: attention (transposed layout) ------
                    with (
                        tc.tile_pool(name="pE", bufs=3) as pE,
                        tc.tile_pool(name="pEs", bufs=2) as pEs,
                        tc.tile_pool(name="psAtt", bufs=2, space="PSUM") as psAtt,
                        tc.tile_pool(name="psAo", bufs=1, space="PSUM") as psAo,
                        tc.tile_pool(name="psDen", bufs=2, space="PSUM") as psDen,
                    ):
                        ps_aoA = psAo.tile([128, 512], F32, tag="aoA")
                        ps_aoB = psAo.tile([128, 512], F32, tag="aoB")
                        for c in range(NCH):
                            ps_den = psDen.tile([128, CL], F32, tag="den")
                            for m in range(NT):
                                ps_a = psAtt.tile([128, H * CL], F32, tag="att")
                                for h in range(H):
                                    nc.tensor.matmul(
                                        ps_a[:, h * CL:(h + 1) * CL],
                                        kT[:, h, m * 128:(m + 1) * 128],
                                        qT[:, h, c * CL:(c + 1) * CL],
                                        start=True, stop=True,
                                    )
                                pT = pE.tile([128, H * CL], BF16, tag="pT")
                                nc.scalar.activation(pT, ps_a, AF.Exp, scale=ATT_SCALE)
                                for h in range(H):
                                    ps_ao = ps_aoA if h < 2 else ps_aoB
                                    # one accumulation group per PSUM bank: the
                                    # bank-wide zero region from the h-even start
                                    # makes the h-odd m==0 write an overwrite.
                                    nc.tensor.matmul(
                                        ps_ao[:, (h % 2) * CL:(h % 2 + 1) * CL],
                                        v_sb[:, m, h * 128:(h + 1) * 128],
                                        pT[:, h * CL:(h + 1) * CL],
                                        start=(m == 0 and h % 2 == 0),
                                        stop=(m == NT - 1 and h % 2 == 1),
                                    )
                                for h in range(H):
                                    nc.tensor.matmul(
                                        ps_den[32 * h:32 * h + 1, :],
                                        ones_col_bf,
                                        pT[:, h * CL:(h + 1) * CL],
                                        start=(m == 0), stop=(m == NT - 1),
                                        tile_position=(0, 32 * h),
                                    )
                            # free the ao banks fast: unnormalized copy to SBUF
                            aoU = pEs.tile([128, H * CL], F32, tag="aoU")
                            nc.vector.tensor_copy(out=aoU[:, 0:2 * CL], in_=ps_aoA)
                            nc.vector.tensor_copy(out=aoU[:, 2 * CL:4 * CL], in_=ps_aoB)
                            # denominators -> 1/den on DVE (keeps the Exp table resident)
                            den_sb = pEs.tile([128, CL], F32, tag="densb")
                            nc.vector.memset(den_sb, 1.0)
                            for h in range(H):
                                nc.vector.tensor_copy(
                                    out=den_sb[32 * h:32 * h + 1, :],
                                    in_=ps_den[32 * h:32 * h + 1, :],
                                )
                            rden_sb = pEs.tile([128, CL], F32, tag="rdensb")
                            nc.vector.reciprocal(rden_sb, den_sb)
                            rden = pEs.tile([1, H * CL], F32, tag="rden")
                            for h in range(H):
                                nc.sync.dma_start(
                                    out=rden[:, h * CL:(h + 1) * CL],
                                    in_=rden_sb[32 * h:32 * h + 1, :],
                                )
                            bc_sb = pEs.tile([128, H * CL], F32, tag="bcsb")
                            nc.gpsimd.partition_broadcast(bc_sb, rden)
                            for h in range(H):
                                nc.vector.scalar_tensor_tensor(
                                    out=aoT[:, h, c * CL:(c + 1) * CL],
                                    in0=aoU[:, h * CL:(h + 1) * CL],
                                    scalar=1.0, in1=bc_sb[:, h * CL:(h + 1) * CL],
                                    op0=ALU.mult, op1=ALU.mult,
                                )

                # ------- phase F: o-proj + residual + layernorm ----------
                with (
                    tc.tile_pool(name="pF", bufs=2) as pF,
                    tc.tile_pool(name="pFbig", bufs=1) as pFbig,
                    tc.tile_pool(name="psF", bufs=2, space="PSUM") as psF,
                ):
                    r_all = pFbig.tile([128, NT, DC], F32)
                    mean_all = pFbig.tile([128, NT], F32)
                    ssq_all = pFbig.tile([128, NT], F32)
                    lnv_all = pFbig.tile([128, NT], F32)
                    rstd_all = pFbig.tile([128, NT], F32)
                    HB = NT // 2
                    for half in range(2):
                        ts0 = half * HB
                        for t in range(ts0, ts0 + HB):
                            ps_o = psF.tile([128, DC], F32, tag="o")
                            for h in range(H):
                                nc.tensor.matmul(
                                    ps_o, aoT[:, h, t * 128:(t + 1) * 128], wo_sb[:, h, :],
                                    start=(h == 0), stop=False,
                                )
                            nc.tensor.matmul(ps_o, ones_row_bf[:, 0:128], bor,
                                             start=False, stop=True)
                            rsum = pF.tile([128, 1], F32, tag="rsum")
                            nc.vector.scalar_tensor_tensor(
                                out=r_all[:, t, :], in0=ps_o, scalar=1.0,
                                in1=cache_sb[:, t, :],
                                op0=ALU.mult, op1=ALU.add, accum_out=rsum,
                            )
                            nc.vector.tensor_scalar_mul(
                                mean_all[:, t:t + 1], rsum, 1.0 / DC)
                            scratch = pF.tile([128, DC], F32, tag="scratch")
                            nc.vector.scalar_tensor_tensor(
                                out=scratch, in0=r_all[:, t, :],
                                scalar=mean_all[:, t:t + 1], in1=r_all[:, t, :],
                                op0=ALU.subtract, op1=ALU.mult,
                                accum_out=ssq_all[:, t:t + 1],
                            )
                        nc.scalar.activation(lnv_all[:, ts0:ts0 + HB],
                                             ssq_all[:, ts0:ts0 + HB], AF.Ln,
                                             scale=1.0 / DC, bias=eps5_t)
                        nc.scalar.activation(rstd_all[:, ts0:ts0 + HB],
                                             lnv_all[:, ts0:ts0 + HB], AF.Exp,
                                             scale=-0.5)
                        for t in range(ts0, ts0 + HB):
                            t1 = pF.tile([128, DC], F32, tag="t1")
                            nc.vector.tensor_scalar(
                                t1, r_all[:, t, :], mean_all[:, t:t + 1],
                                rstd_all[:, t:t + 1], ALU.subtract, ALU.mult)
                            t2 = pF.tile([128, DC], F32, tag="t2")
                            nc.vector.scalar_tensor_tensor(
                                out=t2, in0=t1, scalar=1.0, in1=lng_bc,
                                op0=ALU.mult, op1=ALU.mult,
                            )
                            o_sb = pF.tile([128, DC], F32, tag="osb")
                            nc.vector.scalar_tensor_tensor(
                                out=o_sb, in0=t2, scalar=1.0, in1=lnb_bc,
                                op0=ALU.mult, op1=ALU.add,
                            )
                            nc.sync.dma_start(out=out3[:, t, :], in_=o_sb)

    nc.compile()
    return nc


_NC_CACHE = {}


def _get_nc():
    if "nc" not in _NC_CACHE:
        _NC_CACHE["nc"] = _build()
    return _NC_CACHE["nc"]


def _in_maps(inputs):
    per_batch = {"y", "cache", "gumbel_u"}
    maps = []
    for b in range(B):
        m = {}
        for name in _INPUT_SPECS:
            arr = np.ascontiguousarray(np.asarray(inputs[name], dtype=np.float32))
            m[name] = arr[b] if name in per_batch else arr
        maps.append(m)
    return maps


def _execute(inputs, trace=False):
    nc = _get_nc()
    res = run_bass_kernel_spmd(nc, _in_maps(inputs), list(range(B)), trace=trace)
    out = np.stack([res.results[b]["out"] for b in range(B)]).astype(np.float32)
    return out, res


def kernel(**inputs) -> np.ndarray:
    out, _ = _execute(inputs)
    return out



# revision 24
# speedup vs baseline: 1.0314x; 1.0314x over previous
"""DLSMN scatter-memory + cache self-attention kernel for Trainium2.

Data-parallel over batch: batch b runs on NeuronCore b (8 cores), no
collectives.  Inside one core (one batch):

  phase A: per 128-token tile of y: PE-transpose y (f32r, 1.5cyc/row) ->
           yT chunks, f32r matmuls for write_vals / (logits,gate),
           gumbel-softmax routing (Ln prepass batched first -> exactly 2
           ACT table loads in the whole kernel), weighted-scatter matmuls
           into 2 PSUM banks + separate shared-bank mass accumulators
           (3 banks total, leaving room to pipeline).
  phase B: slot update  upd = (1-g)*DECAY*old + g*updates/(mass+eps).
  phase C: PE-transpose cache2 -> cache2T (fp8e4).
  phase D: q/k/v projections with fp8 DoubleRow matmuls (contraction
           pairs), evacuated with fused bias adds (no bias matmuls):
           qT/kT bf16, v fp8e4.
  phase E: attention transposed: QK^T in bf16 (per 256-col chunk),
           exp on ScalarE -> pT fp8e4 pairs, PV + denominator matmuls in
           fp8 DoubleRow (0.5 cyc/row), softmax normalization fused into
           the PSUM->aoT evacuation.  Software-pipelined so exp overlaps
           QK of the next tile.
  phase F: folded per-chunk into phase E: o-projection in fp8 DoubleRow,
           residual + layernorm with a DVE-only Quake rsqrt (no ACT
           table switches), output DMA per n-tile.
"""

import numpy as np

import concourse.bacc as bacc
import concourse.mybir as mybir
import concourse.tile as tile
from concourse.bass_utils import run_bass_kernel_spmd
from concourse.masks import make_identity

F32 = mybir.dt.float32
F32R = mybir.dt.float32r
F16 = mybir.dt.float16
BF16 = mybir.dt.bfloat16
FP8 = mybir.dt.float8e4
I32 = mybir.dt.int32
AF = mybir.ActivationFunctionType
ALU = mybir.AluOpType
DR = mybir.MatmulPerfMode.DoubleRow

B = 8
S = 2048
D = 1024
DC = 512
K = 256
L = 8
H = 4
HD = 128
N = L * K
LAYER_IDX = 3
DECAY = 0.9
EPS = 1e-6
ST = S // 128   # 16 token tiles
NT = N // 128   # 16 slot tiles
DCH = D // 128  # 8 d_model chunks
CL = 256        # attention n-chunk length
NCH = N // CL   # 8 attention chunks
ATT_SCALE = float(1.0 / np.sqrt(np.float32(HD)))
QMAGIC = 0x5F3759DF + 1  # quake rsqrt magic (+1 for the xor-negate trick)

_INPUT_SPECS = {
    "y": (S, D), "cache": (N, DC), "gumbel_u": (S, K),
    "W_gate": (D, 1), "b_gate": (1,), "W_slot": (D, K), "b_slot": (K,),
    "gamma": (1,), "W_write": (D, DC), "b_write": (DC,),
    "Wq": (DC, DC), "bq": (DC,), "Wk": (DC, DC), "bk": (DC,),
    "Wv": (DC, DC), "bv": (DC,), "Wo": (DC, DC), "bo": (DC,),
    "ln_g": (DC,), "ln_b": (DC,),
}


def _build():
    nc = bacc.Bacc("TRN2", target_bir_lowering=False, debug=False, num_devices=B)

    a = {
        name: nc.dram_tensor(name, list(shape), F32, kind="ExternalInput").ap()
        for name, shape in _INPUT_SPECS.items()
    }
    out_dram = nc.dram_tensor("out", [N, DC], F32, kind="ExternalOutput").ap()

    y3 = a["y"].rearrange("(t p) d -> p t d", p=128)
    gum3 = a["gumbel_u"].rearrange("(t p) k -> p t k", p=128)
    cache3 = a["cache"].rearrange("(t p) d -> p t d", p=128)
    out3 = out_dram.rearrange("(t p) d -> p t d", p=128)

    with tile.TileContext(nc) as tc:
        with (
            tc.tile_pool(name="const", bufs=1) as const,
            tc.tile_pool(name="cachep", bufs=1) as cachep,
            tc.tile_pool(name="attn", bufs=1) as attn,
        ):
            # ---------------- constants ------------------------------------
            ident = const.tile([128, 128], F32)
            make_identity(nc, ident)
            ident_r = const.tile([128, 128], F32R)
            nc.vector.tensor_copy(out=ident_r, in_=ident)
            ones_col_f = const.tile([128, 2], F32)
            nc.vector.memset(ones_col_f, 1.0)
            ones_col_r = const.tile([128, 2], F32R)
            nc.vector.tensor_copy(out=ones_col_r, in_=ones_col_f)
            ones_row_f = const.tile([1, 128], F32)
            nc.vector.memset(ones_row_f, 1.0)
            ones_row_r = const.tile([1, 128], F32R)
            nc.vector.tensor_copy(out=ones_row_r, in_=ones_row_f)
            ones_row_bf = const.tile([1, 128], BF16)
            nc.vector.memset(ones_row_bf, 1.0)
            ones8 = const.tile([128, 1], FP8)
            nc.vector.memset(ones8, 1.0)
            eps8_t = const.tile([128, 1], F32)
            nc.vector.memset(eps8_t, 1e-8)
            gamma_t = const.tile([128, 1], F32)
            nc.sync.dma_start(out=gamma_t, in_=a["gamma"].unsqueeze(0).to_broadcast([128, 1]))
            lng_bc = const.tile([128, DC], F32)
            nc.gpsimd.dma_start(out=lng_bc, in_=a["ln_g"].unsqueeze(0).to_broadcast([128, DC]))
            lnb_bc = const.tile([128, DC], F32)
            nc.gpsimd.dma_start(out=lnb_bc, in_=a["ln_b"].unsqueeze(0).to_broadcast([128, DC]))
            bwr_bc = const.tile([128, DC], F32)
            nc.gpsimd.dma_start(out=bwr_bc, in_=a["b_write"].unsqueeze(0).to_broadcast([128, DC]))
            bv_bc = const.tile([128, DC], F32)
            nc.gpsimd.dma_start(out=bv_bc, in_=a["bv"].unsqueeze(0).to_broadcast([128, DC]))
            # per-partition bias columns for q/k (out partition = dc within head)
            bq_col = const.tile([128, H], F32)
            nc.gpsimd.dma_start(out=bq_col, in_=a["bq"].rearrange("(h p) -> p h", p=128))
            bk_col = const.tile([128, H], F32)
            nc.gpsimd.dma_start(out=bk_col, in_=a["bk"].rearrange("(h p) -> p h", p=128))
            bsg_row = const.tile([1, K + 2], F32R)
            nc.gpsimd.dma_start(out=bsg_row[:, 0:K], in_=a["b_slot"].unsqueeze(0))
            nc.gpsimd.dma_start(out=bsg_row[:, K:K + 1], in_=a["b_gate"].unsqueeze(0))
            nc.gpsimd.dma_start(out=bsg_row[:, K + 1:K + 2], in_=a["b_gate"].unsqueeze(0))
            bor_row = const.tile([1, DC], BF16)
            nc.gpsimd.dma_start(out=bor_row, in_=a["bo"].unsqueeze(0))

            cache_sb = cachep.tile([128, NT, DC], F32R)
            cache_f = cache_sb.bitcast(F32)

            # ---------------- persistent attention tiles -------------------
            c2t = attn.tile([128, 4, N], FP8)
            qT = attn.tile([128, H, N], BF16)
            kT = attn.tile([128, H, N], BF16)
            v_sb = attn.tile([128, NT, DC], FP8)
            wq8 = attn.tile([128, 4, DC], FP8)
            wk8 = attn.tile([128, 4, DC], FP8)
            wv8 = attn.tile([128, 4, DC], FP8)
            wo8 = attn.tile([128, 4, DC], FP8)
            aoT = attn.tile([128, H, N], FP8)

            # ======================= phase A + B ===========================
            with (
                tc.tile_pool(name="wA", bufs=1) as wA,
                tc.tile_pool(name="pA", bufs=2) as pA,
                tc.tile_pool(name="pAs", bufs=3) as pAs,
                tc.tile_pool(name="stageP", bufs=1) as stageP,
                tc.tile_pool(name="psT", bufs=2, space="PSUM") as psT,
                tc.tile_pool(name="psWV", bufs=2, space="PSUM") as psWV,
                tc.tile_pool(name="psLG", bufs=1, space="PSUM") as psLG,
                tc.tile_pool(name="psU", bufs=1, space="PSUM") as psU,
            ):
                # A-phase weights (f32r, straight DMA, no casts)
                wwr = wA.tile([128, DCH, DC], F32R)
                wsg = wA.tile([128, DCH, K + 2], F32R)
                wwr3 = a["W_write"].rearrange("(c p) d -> p c d", p=128)
                wsl3 = a["W_slot"].rearrange("(c p) k -> p c k", p=128)
                nc.gpsimd.dma_start(out=wwr, in_=wwr3)
                nc.gpsimd.dma_start(out=wsg[:, :, 0:K], in_=wsl3)
                nc.gpsimd.dma_start(out=wsg[:, :, K:K + 1], in_=a["W_gate"].rearrange("(c p) o -> p c o", p=128))
                nc.gpsimd.dma_start(out=wsg[:, :, K + 1:K + 2], in_=a["W_gate"].rearrange("(c p) o -> p c o", p=128))
                # cache for phases B/C/F
                nc.gpsimd.dma_start(out=cache_sb, in_=cache3.bitcast(F32R))
                # attention weights: DMA fp32 staging -> cast on gpsimd
                stage = stageP.tile([128, 4, DC], F32, tag="stage")
                wq3 = a["Wq"].rearrange("(c p) d -> p c d", p=128)
                wk3 = a["Wk"].rearrange("(c p) d -> p c d", p=128)
                wv3 = a["Wv"].rearrange("(c p) d -> p c d", p=128)
                wo3 = a["Wo"].rearrange("(c p) d -> p c d", p=128)
                for src, dst in ((wq3, wq8), (wk3, wk8), (wv3, wv8), (wo3, wo8)):
                    nc.gpsimd.dma_start(out=stage, in_=src)
                    nc.gpsimd.tensor_copy(out=dst, in_=stage)

                # gumbel Ln prepass: all Ln ops batched (one ACT table set)
                lnz_all = wA.tile([128, ST, K], F16)
                for g in range(4):
                    gum = pA.tile([128, 4, K], F32, tag="gum")
                    nc.sync.dma_start(out=gum, in_=gum3[:, 4 * g:4 * g + 4, :])
                    lnu = pA.tile([128, 4, K], F32, tag="lnu")
                    nc.scalar.activation(lnu, gum, AF.Ln, bias=eps8_t)
                    nc.scalar.activation(lnz_all[:, 4 * g:4 * g + 4, :], lnu, AF.Ln,
                                         bias=eps8_t, scale=-1.0)

                # persistent scatter accumulators:
                #   updates: one bank per kc (512 fp32)
                #   mass: single shared bank [128, 2] (col = kc); the kc=1
                #   group starts with start=False into the zero-region left
                #   by kc=0's bank-wide start (overwrite via has_written).
                ps_upd = [psU.tile([128, DC], F32, name=f"upd{kc}", tag=f"upd{kc}")
                          for kc in range(2)]
                ps_mass = psU.tile([128, 4], F32, name="mass", tag="mass")

                prev = [None]

                def flush_scatter():
                    if prev[0] is None:
                        return
                    j, w_j, wv_j = prev[0]
                    for kc in range(2):
                        lhs = w_j[:, kc * 128:(kc + 1) * 128]
                        nc.tensor.matmul(ps_upd[kc], lhs, wv_j,
                                         start=(j == 0), stop=(j == ST - 1))
                        nc.tensor.matmul(ps_mass[:, 2 * kc:2 * kc + 2], lhs,
                                         ones_col_r,
                                         start=(j == 0 and kc == 0),
                                         stop=(j == ST - 1 and kc == 1),
                                         skip_group_check=True)
                    prev[0] = None

                for i in range(ST):
                    y_t = pA.tile([128, D], F32R, tag="y")
                    nc.sync.dma_start(out=y_t, in_=y3[:, i, :].bitcast(F32R))

                    # transpose y tile -> yT (f32r: 1.5 cyc/row)
                    yT = pA.tile([128, D], F32R, tag="yT")
                    for g in range(2):
                        tr = psT.tile([128, 512], F32R, tag="tr")
                        for cc in range(4):
                            c = 4 * g + cc
                            nc.tensor.transpose(
                                tr[:, cc * 128:(cc + 1) * 128],
                                y_t[:, c * 128:(c + 1) * 128],
                                ident_r,
                            )
                        nc.vector.tensor_copy(out=yT[:, g * 512:(g + 1) * 512],
                                              in_=tr.bitcast(F32))

                    # write_vals / (logits, gate) matmuls
                    ps_wv = psWV.tile([128, DC], F32, tag="wv")
                    for c in range(DCH):
                        nc.tensor.matmul(
                            ps_wv, yT[:, c * 128:(c + 1) * 128], wwr[:, c, :],
                            start=(c == 0), stop=(c == DCH - 1),
                        )
                    ps_lg = psLG.tile([128, K + 2], F32, tag="lg")
                    for c in range(DCH):
                        nc.tensor.matmul(
                            ps_lg, yT[:, c * 128:(c + 1) * 128], wsg[:, c, :],
                            start=(c == 0), stop=False,
                        )
                    nc.tensor.matmul(ps_lg, ones_row_r, bsg_row,
                                     start=False, stop=True)

                    # scatter matmuls for the previous tile (keeps PE dense
                    # while this tile's DVE/ACT chain runs)
                    flush_scatter()

                    # t = gamma*logits - lnz
                    t_sb = pAs.tile([128, K], F32, tag="tsb")
                    nc.vector.scalar_tensor_tensor(
                        out=t_sb, in0=ps_lg[:, 0:K], scalar=gamma_t,
                        in1=lnz_all[:, i, :], op0=ALU.mult, op1=ALU.subtract,
                    )
                    # scores = sigmoid(gate)
                    sc_e = pAs.tile([128, 1], F32, tag="sce")
                    nc.scalar.activation(sc_e, ps_lg[:, K:K + 1], AF.Exp, scale=-1.0)
                    sc1 = pAs.tile([128, 1], F32, tag="sc1")
                    nc.vector.tensor_scalar_add(sc1, sc_e, 1.0)
                    scores = pAs.tile([128, 1], F32, tag="scores")
                    nc.vector.reciprocal(scores, sc1)
                    # p_unnorm = exp(t) with fused row-sum
                    p_un = pAs.tile([128, K], F32, tag="pun")
                    rs = pAs.tile([128, 1], F32, tag="rs")
                    nc.scalar.activation(p_un, t_sb, AF.Exp, accum_out=rs)
                    rrs = pAs.tile([128, 1], F32, tag="rrs")
                    nc.vector.reciprocal(rrs, rs)
                    s2 = pAs.tile([128, 1], F32, tag="s2")
                    nc.vector.tensor_tensor(s2, scores, rrs, ALU.mult)
                    w_sb = pAs.tile([128, K], F32R, tag="wsb")
                    nc.vector.tensor_scalar_mul(w_sb, p_un, s2)
                    # wv_sb = write_vals + b_write
                    wv_sb = pAs.tile([128, DC], F32R, tag="wvsb")
                    nc.vector.scalar_tensor_tensor(
                        out=wv_sb, in0=ps_wv, scalar=1.0,
                        in1=bwr_bc, op0=ALU.mult, op1=ALU.add,
                    )
                    prev[0] = (i, w_sb, wv_sb)

                flush_scatter()

                # ------- phase B: slot update, overwrite cache rows -------
                base_t = LAYER_IDX * K // 128  # n-tile 6
                for kc in range(2):
                    mass = pAs.tile([128, 1], F32, tag="mass")
                    nc.vector.tensor_copy(out=mass, in_=ps_mass[:, 2 * kc:2 * kc + 1])
                    m1 = pAs.tile([128, 1], F32, tag="m1")
                    nc.vector.tensor_scalar_add(m1, mass, EPS)
                    rm = pAs.tile([128, 1], F32, tag="rm")
                    nc.vector.reciprocal(rm, m1)
                    m2 = pAs.tile([128, 1], F32, tag="m2")
                    nc.vector.tensor_scalar_add(m2, mass, 1.0)
                    rg = pAs.tile([128, 1], F32, tag="rg")
                    nc.vector.reciprocal(rg, m2)
                    g_t = pAs.tile([128, 1], F32, tag="gt")
                    nc.vector.tensor_tensor(g_t, mass, rg, ALU.mult)
                    co = pAs.tile([128, 1], F32, tag="co")
                    nc.vector.tensor_scalar(co, g_t, -DECAY, DECAY, ALU.mult, ALU.add)
                    cn = pAs.tile([128, 1], F32, tag="cn")
                    nc.vector.tensor_tensor(cn, g_t, rm, ALU.mult)

                    told = pAs.tile([128, DC], F32, tag="told")
                    nc.vector.tensor_scalar_mul(told, cache_f[:, base_t + kc, :], co)
                    nc.vector.scalar_tensor_tensor(
                        out=cache_sb[:, base_t + kc, :],
                        in0=ps_upd[kc], scalar=cn, in1=told,
                        op0=ALU.mult, op1=ALU.add,
                    )

            # ======================= phases C + D ==========================
            with (
                tc.tile_pool(name="psC", bufs=2, space="PSUM") as psC,
                tc.tile_pool(name="psD", bufs=3, space="PSUM") as psD,
            ):
                # ------- phase C: cache2 -> cache2T (fp8) ------------------
                evac_flip = [0]

                def evac(out_ap, in_ap, scalar_copy, stt_in1=None, ts_col=None):
                    """PSUM->SBUF evacuation, alternating scalar/vector."""
                    use_scalar = scalar_copy and (evac_flip[0] % 2 == 0)
                    evac_flip[0] += 1
                    if stt_in1 is not None:
                        nc.vector.scalar_tensor_tensor(
                            out=out_ap, in0=in_ap, scalar=1.0, in1=stt_in1,
                            op0=ALU.mult, op1=ALU.add)
                    elif ts_col is not None:
                        nc.vector.tensor_scalar_add(out_ap, in_ap, ts_col)
                    elif use_scalar:
                        nc.scalar.copy(out=out_ap, in_=in_ap)
                    else:
                        nc.vector.tensor_copy(out=out_ap, in_=in_ap)

                for j in range(4):
                    for tg in range(4):
                        ps = psC.tile([128, 512], F32R, tag="ctr")
                        for tt in range(4):
                            t = tg * 4 + tt
                            nc.tensor.transpose(
                                ps[:, tt * 128:(tt + 1) * 128],
                                cache_sb[:, t, j * 128:(j + 1) * 128],
                                ident_r,
                            )
                        evac(c2t[:, j, tg * 512:(tg + 1) * 512], ps.bitcast(F32),
                             scalar_copy=True)

                # ------- phase D: projections (fp8 DoubleRow) --------------
                # v first (needed early in E), then k, then q (chunk order)
                for m in range(NT):
                    ps = psD.tile([128, DC], F32, tag="v")
                    for g in range(2):
                        nc.tensor.matmul(
                            ps, c2t[:, 2 * g:2 * g + 2, m * 128:(m + 1) * 128],
                            wv8[:, 2 * g:2 * g + 2, :],
                            start=(g == 0), stop=(g == 1), perf_mode=DR,
                        )
                    evac(v_sb[:, m, :], ps, scalar_copy=False, stt_in1=bv_bc)
                for dst, w8, b_col in ((kT, wk8, bk_col), (qT, wq8, bq_col)):
                    for c in range(4):
                        for h in range(H):
                            ps = psD.tile([128, 512], F32, tag="qk")
                            for g in range(2):
                                nc.tensor.matmul(
                                    ps, w8[:, 2 * g:2 * g + 2, h * 128:(h + 1) * 128],
                                    c2t[:, 2 * g:2 * g + 2, c * 512:(c + 1) * 512],
                                    start=(g == 0), stop=(g == 1), perf_mode=DR,
                                )
                            evac(dst[:, h, c * 512:(c + 1) * 512], ps,
                                 scalar_copy=False, ts_col=b_col[:, h:h + 1])

            # ======================= phases E + F ==========================
            with (
                tc.tile_pool(name="pE", bufs=2) as pE,
                tc.tile_pool(name="pEs", bufs=2) as pEs,
                tc.tile_pool(name="pF", bufs=2) as pF,
                tc.tile_pool(name="psAtt", bufs=2, space="PSUM") as psAtt,
                tc.tile_pool(name="psAo", bufs=1, space="PSUM") as psAo,
                tc.tile_pool(name="psDen", bufs=1, space="PSUM") as psDen,
                tc.tile_pool(name="psF", bufs=1, space="PSUM") as psF,
            ):
                for c in range(NCH):
                    ps_aoA = psAo.tile([128, 512], F32, name=f"aoA{c}", tag="aoA")
                    ps_aoB = psAo.tile([128, 512], F32, name=f"aoB{c}", tag="aoB")
                    ps_den = psDen.tile([128, CL], F32, tag="den")
                    pTp = None
                    for m in range(NT):
                        ps_a = psAtt.tile([128, H * CL], F32, tag="att")
                        for h in range(H):
                            nc.tensor.matmul(
                                ps_a[:, h * CL:(h + 1) * CL],
                                kT[:, h, m * 128:(m + 1) * 128],
                                qT[:, h, c * CL:(c + 1) * CL],
                                start=True, stop=True,
                            )
                        if m % 2 == 0:
                            pTp = pE.tile([128, H, 2, CL], FP8, tag="pT")
                        nc.scalar.activation(
                            pTp[:, :, m % 2, :],
                            ps_a.rearrange("p (h n) -> p h n", h=H),
                            AF.Exp, scale=ATT_SCALE)
                        if m % 2 == 1:
                            j = m // 2
                            for h in range(H):
                                ps_ao = ps_aoA if h < 2 else ps_aoB
                                nc.tensor.matmul(
                                    ps_ao[:, (h % 2) * CL:(h % 2 + 1) * CL],
                                    v_sb[:, m - 1:m + 1, h * 128:(h + 1) * 128],
                                    pTp[:, h, :, :],
                                    start=(j == 0 and h % 2 == 0),
                                    stop=(j == NCH - 1 and h % 2 == 1),
                                    perf_mode=DR,
                                )
                            for j2 in range(2):
                                for h in range(H):
                                    nc.tensor.matmul(
                                        ps_den[32 * h:32 * h + 1, :],
                                        ones8,
                                        pTp[:, h, j2, :],
                                        start=(j == 0 and j2 == 0),
                                        stop=(j == NCH - 1 and j2 == 1),
                                        tile_position=(0, 32 * h),
                                    )
                    # denominators -> 1/den, broadcast, fused normalize evac
                    den_sb = pEs.tile([128, CL], F32, tag="densb")
                    nc.vector.memset(den_sb, 1.0)
                    for h in range(H):
                        nc.vector.tensor_copy(
                            out=den_sb[32 * h:32 * h + 1, :],
                            in_=ps_den[32 * h:32 * h + 1, :],
                        )
                    rden_sb = pEs.tile([128, CL], F32, tag="rdensb")
                    nc.vector.reciprocal(rden_sb, den_sb)
                    rden = pEs.tile([1, H * CL], F32, tag="rden")
                    for h in range(H):
                        nc.sync.dma_start(
                            out=rden[:, h * CL:(h + 1) * CL],
                            in_=rden_sb[32 * h:32 * h + 1, :],
                        )
                    bc_sb = pEs.tile([128, H * CL], F32, tag="bcsb")
                    nc.gpsimd.partition_broadcast(bc_sb, rden)
                    for h in range(H):
                        ps_ao = ps_aoA if h < 2 else ps_aoB
                        nc.vector.scalar_tensor_tensor(
                            out=aoT[:, h, c * CL:(c + 1) * CL],
                            in0=ps_ao[:, (h % 2) * CL:(h % 2 + 1) * CL],
                            scalar=1.0, in1=bc_sb[:, h * CL:(h + 1) * CL],
                            op0=ALU.mult, op1=ALU.mult,
                        )

                    # ------- phase F for n-tiles 2c, 2c+1 ------------------
                    mean2 = pF.tile([128, 2], F32, tag="mean2")
                    var2 = pF.tile([128, 2], F32, tag="var2")
                    r_ts = []
                    for half in range(2):
                        t = 2 * c + half
                        ps_o = psF.tile([128, DC], F32, tag="o")
                        for g in range(2):
                            nc.tensor.matmul(
                                ps_o, aoT[:, 2 * g:2 * g + 2, t * 128:(t + 1) * 128],
                                wo8[:, 2 * g:2 * g + 2, :],
                                start=(g == 0), stop=False, perf_mode=DR,
                            )
                        nc.tensor.matmul(ps_o, ones_row_bf, bor_row,
                                         start=False, stop=True)
                        r_t = pF.tile([128, DC], F32, tag="r")
                        rsum = pF.tile([128, 1], F32, tag="rsum")
                        nc.vector.scalar_tensor_tensor(
                            out=r_t, in0=ps_o, scalar=1.0,
                            in1=cache_f[:, t, :],
                            op0=ALU.mult, op1=ALU.add, accum_out=rsum,
                        )
                        nc.vector.tensor_scalar_mul(
                            mean2[:, half:half + 1], rsum, 1.0 / DC)
                        scratch = pF.tile([128, DC], F32, tag="scratch")
                        nc.vector.scalar_tensor_tensor(
                            out=scratch, in0=r_t,
                            scalar=mean2[:, half:half + 1], in1=r_t,
                            op0=ALU.subtract, op1=ALU.mult,
                            accum_out=var2[:, half:half + 1],
                        )
                        r_ts.append(r_t)
                    # rstd = 1/sqrt(var/DC + 1e-5), quake + 2 Newton (DVE only)
                    nc.vector.tensor_scalar(var2, var2, 1.0 / DC, 1e-5,
                                            ALU.mult, ALU.add)
                    vh = pF.tile([128, 2], F32, tag="vh")
                    nc.vector.tensor_scalar_mul(vh, var2, -0.5)
                    qi = pF.tile([128, 2], I32, tag="qi")
                    nc.vector.tensor_scalar(qi, var2.bitcast(I32), 1, -1,
                                            ALU.logical_shift_right, ALU.bitwise_xor)
                    rstd = pF.tile([128, 2], F32, tag="rstd")
                    nc.vector.tensor_scalar_add(rstd.bitcast(I32), qi, QMAGIC)
                    yy = pF.tile([128, 2], F32, tag="yy")
                    for _ in range(2):
                        nc.vector.tensor_tensor(yy, rstd, rstd, ALU.mult)
                        nc.vector.tensor_tensor(yy, yy, vh, ALU.mult)
                        nc.vector.tensor_scalar_add(yy, yy, 1.5)
                        nc.vector.tensor_tensor(rstd, rstd, yy, ALU.mult)
                    for half in range(2):
                        t = 2 * c + half
                        t1 = pF.tile([128, DC], F32, tag="t1")
                        nc.vector.tensor_scalar(
                            t1, r_ts[half], mean2[:, half:half + 1],
                            rstd[:, half:half + 1], ALU.subtract, ALU.mult)
                        t2 = pF.tile([128, DC], F32, tag="t2")
                        nc.vector.scalar_tensor_tensor(
                            out=t2, in0=t1, scalar=1.0, in1=lng_bc,
                            op0=ALU.mult, op1=ALU.mult,
                        )
                        o_sb = pF.tile([128, DC], F32, tag="osb")
                        nc.gpsimd.tensor_tensor(o_sb, t2, lnb_bc, ALU.add)
                        nc.sync.dma_start(out=out3[:, t, :], in_=o_sb)

    nc.compile()
    return nc


_NC_CACHE = {}


def _get_nc():
    if "nc" not in _NC_CACHE:
        _NC_CACHE["nc"] = _build()
    return _NC_CACHE["nc"]


def _in_maps(inputs):
    per_batch = {"y", "cache", "gumbel_u"}
    maps = []
    for b in range(B):
        m = {}
        for name in _INPUT_SPECS:
            arr = np.ascontiguousarray(np.asarray(inputs[name], dtype=np.float32))
            m[name] = arr[b] if name in per_batch else arr
        maps.append(m)
    return maps


def _execute(inputs, trace=False):
    nc = _get_nc()
    res = run_bass_kernel_spmd(nc, _in_maps(inputs), list(range(B)), trace=trace)
    out = np.stack([res.results[b]["out"] for b in range(B)]).astype(np.float32)
    return out, res


def kernel(**inputs) -> np.ndarray:
    out, _ = _execute(inputs)
    return out


# revision 25
# speedup vs baseline: 1.3863x; 1.3441x over previous
"""DLSMN scatter-memory + cache self-attention kernel for Trainium2.

Data-parallel over batch: batch b runs on NeuronCore b (8 cores), no
collectives.  Inside one core (one batch):

  phase A: per 128-token tile of y: PE-transpose y (f32r transposes) ->
           yT (bf16), bf16 matmuls (FWL weight loads) for write_vals /
           (logits,gate), gumbel-softmax routing (all Ln batched first ->
           2 ACT table loads total), weighted-scatter matmuls into 2 PSUM
           banks + a shared mass bank (3 banks total).
  phase B: slot update  upd = (1-g)*DECAY*old + g*updates/(mass+eps).
  phase C: PE-transpose cache2 -> cache2T (fp8e4).
  phase D: q/k/v projections with fp8 DoubleRow matmuls, bias folded
           into the PSUM evacuations: qT/kT bf16, v fp8e4.
  phase E: attention transposed, QK^T bf16.  Chunk structure: all 16 QK
           tiles first (exp on ScalarE streams into a persistent fp8 pT
           buffer), then the o-projection of the PREVIOUS chunk, then PV
           (fp8 DoubleRow) + denominator matmuls.  The previous chunk's
           softmax-normalization tail overlaps this chunk's QK segment.
  phase F: pipelined one chunk behind phase E: o-projection in fp8
           DoubleRow, residual + layernorm with a DVE-only Quake rsqrt,
           output DMA per n-tile.
"""

import numpy as np

import concourse.bacc as bacc
import concourse.mybir as mybir
import concourse.tile as tile
from concourse.bass_utils import run_bass_kernel_spmd
from concourse.masks import make_identity

F32 = mybir.dt.float32
F32R = mybir.dt.float32r
F16 = mybir.dt.float16
BF16 = mybir.dt.bfloat16
FP8 = mybir.dt.float8e4
I32 = mybir.dt.int32
AF = mybir.ActivationFunctionType
ALU = mybir.AluOpType
DR = mybir.MatmulPerfMode.DoubleRow

B = 8
S = 2048
D = 1024
DC = 512
K = 256
L = 8
H = 4
HD = 128
N = L * K
LAYER_IDX = 3
DECAY = 0.9
EPS = 1e-6
ST = S // 128   # 16 token tiles
NT = N // 128   # 16 slot tiles
DCH = D // 128  # 8 d_model chunks
CL = 256        # attention n-chunk length
NCH = N // CL   # 8 attention chunks
ATT_SCALE = float(1.0 / np.sqrt(np.float32(HD)))
QMAGIC = 0x5F3759DF + 1  # quake rsqrt magic (+1 for the xor-negate trick)

_INPUT_SPECS = {
    "y": (S, D), "cache": (N, DC), "gumbel_u": (S, K),
    "W_gate": (D, 1), "b_gate": (1,), "W_slot": (D, K), "b_slot": (K,),
    "gamma": (1,), "W_write": (D, DC), "b_write": (DC,),
    "Wq": (DC, DC), "bq": (DC,), "Wk": (DC, DC), "bk": (DC,),
    "Wv": (DC, DC), "bv": (DC,), "Wo": (DC, DC), "bo": (DC,),
    "ln_g": (DC,), "ln_b": (DC,),
}


def _build():
    nc = bacc.Bacc("TRN2", target_bir_lowering=False, debug=False, num_devices=B)

    a = {
        name: nc.dram_tensor(name, list(shape), F32, kind="ExternalInput").ap()
        for name, shape in _INPUT_SPECS.items()
    }
    out_dram = nc.dram_tensor("out", [N, DC], F32, kind="ExternalOutput").ap()

    y3 = a["y"].rearrange("(t p) d -> p t d", p=128)
    gum3 = a["gumbel_u"].rearrange("(t p) k -> p t k", p=128)
    cache3 = a["cache"].rearrange("(t p) d -> p t d", p=128)
    out3 = out_dram.rearrange("(t p) d -> p t d", p=128)

    with tile.TileContext(nc) as tc:
        with (
            tc.tile_pool(name="const", bufs=1) as const,
            tc.tile_pool(name="cachep", bufs=1) as cachep,
            tc.tile_pool(name="attn", bufs=1) as attn,
        ):
            # ---------------- constants ------------------------------------
            ident = const.tile([128, 128], F32)
            make_identity(nc, ident)
            ident_r = const.tile([128, 128], F32R)
            nc.vector.tensor_copy(out=ident_r, in_=ident)
            ones_row_bf = const.tile([1, 128], BF16)
            nc.vector.memset(ones_row_bf, 1.0)
            ones_col2_bf = const.tile([128, 2], BF16)
            nc.vector.memset(ones_col2_bf, 1.0)
            ones8 = const.tile([128, 1], FP8)
            nc.vector.memset(ones8, 1.0)
            eps8_t = const.tile([128, 1], F32)
            nc.vector.memset(eps8_t, 1e-8)
            gamma_t = const.tile([128, 1], F32)
            nc.sync.dma_start(out=gamma_t, in_=a["gamma"].unsqueeze(0).to_broadcast([128, 1]))
            lng_bc = const.tile([128, DC], F32)
            nc.gpsimd.dma_start(out=lng_bc, in_=a["ln_g"].unsqueeze(0).to_broadcast([128, DC]))
            lnb_bc = const.tile([128, DC], F32)
            nc.gpsimd.dma_start(out=lnb_bc, in_=a["ln_b"].unsqueeze(0).to_broadcast([128, DC]))
            bwr_bc = const.tile([128, DC], F32)
            nc.gpsimd.dma_start(out=bwr_bc, in_=a["b_write"].unsqueeze(0).to_broadcast([128, DC]))
            bv_bc = const.tile([128, DC], F32)
            nc.gpsimd.dma_start(out=bv_bc, in_=a["bv"].unsqueeze(0).to_broadcast([128, DC]))
            # per-partition bias columns for q/k (out partition = dc within head)
            bq_col = const.tile([128, H], F32)
            nc.gpsimd.dma_start(out=bq_col, in_=a["bq"].rearrange("(h p) -> p h", p=128))
            bk_col = const.tile([128, H], F32)
            nc.gpsimd.dma_start(out=bk_col, in_=a["bk"].rearrange("(h p) -> p h", p=128))
            bsg_row = const.tile([1, K + 2], BF16)
            nc.gpsimd.dma_start(out=bsg_row[:, 0:K], in_=a["b_slot"].unsqueeze(0))
            nc.gpsimd.dma_start(out=bsg_row[:, K:K + 1], in_=a["b_gate"].unsqueeze(0))
            nc.gpsimd.dma_start(out=bsg_row[:, K + 1:K + 2], in_=a["b_gate"].unsqueeze(0))
            bor_row = const.tile([1, DC], BF16)
            nc.gpsimd.dma_start(out=bor_row, in_=a["bo"].unsqueeze(0))

            cache_sb = cachep.tile([128, NT, DC], F32R)
            cache_f = cache_sb.bitcast(F32)

            # ---------------- persistent attention tiles -------------------
            c2t = attn.tile([128, 4, N], FP8)
            qT = attn.tile([128, H, N], BF16)
            kT = attn.tile([128, H, N], BF16)
            v_sb = attn.tile([128, NT, DC], FP8)
            wq8 = attn.tile([128, 4, DC], FP8)
            wk8 = attn.tile([128, 4, DC], FP8)
            wv8 = attn.tile([128, 4, DC], FP8)
            wo8 = attn.tile([128, 4, DC], FP8)
            aoT = attn.tile([128, H, N], FP8)
            pT = attn.tile([128, H, NT, CL], FP8)

            # ======================= phase A + B ===========================
            with (
                tc.tile_pool(name="wA", bufs=1) as wA,
                tc.tile_pool(name="pA", bufs=2) as pA,
                tc.tile_pool(name="pAs", bufs=3) as pAs,
                tc.tile_pool(name="psT", bufs=2, space="PSUM") as psT,
                tc.tile_pool(name="psWV", bufs=2, space="PSUM") as psWV,
                tc.tile_pool(name="psLG", bufs=1, space="PSUM") as psLG,
                tc.tile_pool(name="psU", bufs=1, space="PSUM") as psU,
            ):
                # weights: casting DMAs on the gpsimd queue
                wwr = wA.tile([128, DCH, DC], BF16)
                wsg = wA.tile([128, DCH, K + 2], BF16)
                wwr3 = a["W_write"].rearrange("(c p) d -> p c d", p=128)
                wsl3 = a["W_slot"].rearrange("(c p) k -> p c k", p=128)
                nc.gpsimd.dma_start(out=wwr, in_=wwr3)
                nc.gpsimd.dma_start(out=wsg[:, :, 0:K], in_=wsl3)
                nc.gpsimd.dma_start(out=wsg[:, :, K:K + 1], in_=a["W_gate"].rearrange("(c p) o -> p c o", p=128))
                nc.gpsimd.dma_start(out=wsg[:, :, K + 1:K + 2], in_=a["W_gate"].rearrange("(c p) o -> p c o", p=128))
                # cache for phases B/C/F
                nc.gpsimd.dma_start(out=cache_sb, in_=cache3.bitcast(F32R))
                # attention weights (fp8/bf16 casting DMAs)
                wq3 = a["Wq"].rearrange("(c p) d -> p c d", p=128)
                wk3 = a["Wk"].rearrange("(c p) d -> p c d", p=128)
                wv3 = a["Wv"].rearrange("(c p) d -> p c d", p=128)
                wo3 = a["Wo"].rearrange("(c p) d -> p c d", p=128)
                nc.gpsimd.dma_start(out=wq8, in_=wq3)
                nc.gpsimd.dma_start(out=wk8, in_=wk3)
                nc.gpsimd.dma_start(out=wv8, in_=wv3)
                nc.gpsimd.dma_start(out=wo8, in_=wo3)

                # gumbel Ln prepass: all Ln ops batched (one ACT table set)
                lnz_all = wA.tile([128, ST, K], F16)
                for g in range(4):
                    gum = pA.tile([128, 4, K], F32, tag="gum")
                    nc.sync.dma_start(out=gum, in_=gum3[:, 4 * g:4 * g + 4, :])
                    lnu = pA.tile([128, 4, K], F32, tag="lnu")
                    nc.scalar.activation(lnu, gum, AF.Ln, bias=eps8_t)
                    nc.scalar.activation(lnz_all[:, 4 * g:4 * g + 4, :], lnu, AF.Ln,
                                         bias=eps8_t, scale=-1.0)

                # persistent scatter accumulators:
                #   updates: one bank per kc (512 fp32)
                #   mass: single shared bank [128, 4] (cols 2kc:2kc+2)
                ps_upd = [psU.tile([128, DC], F32, name=f"upd{kc}", tag=f"upd{kc}")
                          for kc in range(2)]
                ps_mass = psU.tile([128, 4], F32, name="mass", tag="mass")

                prev = [None]

                def flush_scatter():
                    if prev[0] is None:
                        return
                    j, w_j, wv_j = prev[0]
                    for kc in range(2):
                        lhs = w_j[:, kc * 128:(kc + 1) * 128]
                        nc.tensor.matmul(ps_upd[kc], lhs, wv_j,
                                         start=(j == 0), stop=(j == ST - 1))
                        nc.tensor.matmul(ps_mass[:, 2 * kc:2 * kc + 2], lhs,
                                         ones_col2_bf,
                                         start=(j == 0 and kc == 0),
                                         stop=(j == ST - 1 and kc == 1),
                                         skip_group_check=True)
                    prev[0] = None

                for i in range(ST):
                    y_t = pA.tile([128, D], F32R, tag="y")
                    nc.sync.dma_start(out=y_t, in_=y3[:, i, :].bitcast(F32R))

                    # transpose y tile -> yT (f32r transposes, bf16 evac)
                    yT = pA.tile([128, D], BF16, tag="yT")
                    for g in range(2):
                        tr = psT.tile([128, 512], F32R, tag="tr")
                        for cc in range(4):
                            c = 4 * g + cc
                            nc.tensor.transpose(
                                tr[:, cc * 128:(cc + 1) * 128],
                                y_t[:, c * 128:(c + 1) * 128],
                                ident_r,
                            )
                        nc.vector.tensor_copy(out=yT[:, g * 512:(g + 1) * 512],
                                              in_=tr.bitcast(F32))

                    # write_vals / (logits, gate) matmuls
                    ps_wv = psWV.tile([128, DC], F32, tag="wv")
                    for c in range(DCH):
                        nc.tensor.matmul(
                            ps_wv, yT[:, c * 128:(c + 1) * 128], wwr[:, c, :],
                            start=(c == 0), stop=(c == DCH - 1),
                        )
                    ps_lg = psLG.tile([128, K + 2], F32, tag="lg")
                    for c in range(DCH):
                        nc.tensor.matmul(
                            ps_lg, yT[:, c * 128:(c + 1) * 128], wsg[:, c, :],
                            start=(c == 0), stop=False,
                        )
                    nc.tensor.matmul(ps_lg, ones_row_bf, bsg_row,
                                     start=False, stop=True)

                    # scatter matmuls for the previous tile (keeps PE dense
                    # while this tile's DVE/ACT chain runs)
                    flush_scatter()

                    # t = gamma*logits - lnz
                    t_sb = pAs.tile([128, K], F32, tag="tsb")
                    nc.vector.scalar_tensor_tensor(
                        out=t_sb, in0=ps_lg[:, 0:K], scalar=gamma_t,
                        in1=lnz_all[:, i, :], op0=ALU.mult, op1=ALU.subtract,
                    )
                    # scores = sigmoid(gate)
                    sc_e = pAs.tile([128, 1], F32, tag="sce")
                    nc.scalar.activation(sc_e, ps_lg[:, K:K + 1], AF.Exp, scale=-1.0)
                    sc1 = pAs.tile([128, 1], F32, tag="sc1")
                    nc.vector.tensor_scalar_add(sc1, sc_e, 1.0)
                    scores = pAs.tile([128, 1], F32, tag="scores")
                    nc.vector.reciprocal(scores, sc1)
                    # p_unnorm = exp(t) with fused row-sum
                    p_un = pAs.tile([128, K], F32, tag="pun")
                    rs = pAs.tile([128, 1], F32, tag="rs")
                    nc.scalar.activation(p_un, t_sb, AF.Exp, accum_out=rs)
                    rrs = pAs.tile([128, 1], F32, tag="rrs")
                    nc.vector.reciprocal(rrs, rs)
                    s2 = pAs.tile([128, 1], F32, tag="s2")
                    nc.vector.tensor_tensor(s2, scores, rrs, ALU.mult)
                    w_sb = pAs.tile([128, K], BF16, tag="wsb")
                    nc.vector.tensor_scalar_mul(w_sb, p_un, s2)
                    # wv_sb = write_vals + b_write
                    wv_sb = pAs.tile([128, DC], BF16, tag="wvsb")
                    nc.vector.scalar_tensor_tensor(
                        out=wv_sb, in0=ps_wv, scalar=1.0,
                        in1=bwr_bc, op0=ALU.mult, op1=ALU.add,
                    )
                    prev[0] = (i, w_sb, wv_sb)

                flush_scatter()

                # ------- phase B: slot update, overwrite cache rows -------
                base_t = LAYER_IDX * K // 128  # n-tile 6
                for kc in range(2):
                    mass = pAs.tile([128, 1], F32, tag="mass")
                    nc.vector.tensor_copy(out=mass, in_=ps_mass[:, 2 * kc:2 * kc + 1])
                    m1 = pAs.tile([128, 1], F32, tag="m1")
                    nc.vector.tensor_scalar_add(m1, mass, EPS)
                    rm = pAs.tile([128, 1], F32, tag="rm")
                    nc.vector.reciprocal(rm, m1)
                    m2 = pAs.tile([128, 1], F32, tag="m2")
                    nc.vector.tensor_scalar_add(m2, mass, 1.0)
                    rg = pAs.tile([128, 1], F32, tag="rg")
                    nc.vector.reciprocal(rg, m2)
                    g_t = pAs.tile([128, 1], F32, tag="gt")
                    nc.vector.tensor_tensor(g_t, mass, rg, ALU.mult)
                    co = pAs.tile([128, 1], F32, tag="co")
                    nc.vector.tensor_scalar(co, g_t, -DECAY, DECAY, ALU.mult, ALU.add)
                    cn = pAs.tile([128, 1], F32, tag="cn")
                    nc.vector.tensor_tensor(cn, g_t, rm, ALU.mult)

                    told = pAs.tile([128, DC], F32, tag="told")
                    nc.vector.tensor_scalar_mul(told, cache_f[:, base_t + kc, :], co)
                    nc.vector.scalar_tensor_tensor(
                        out=cache_sb[:, base_t + kc, :],
                        in0=ps_upd[kc], scalar=cn, in1=told,
                        op0=ALU.mult, op1=ALU.add,
                    )

            # ======================= phases C + D ==========================
            with (
                tc.tile_pool(name="psC", bufs=2, space="PSUM") as psC,
                tc.tile_pool(name="psD", bufs=3, space="PSUM") as psD,
            ):
                # ------- phase C: cache2 -> cache2T (fp8) ------------------
                evac_flip = [0]

                def evac_copy(out_ap, in_ap):
                    if evac_flip[0] % 2 == 0:
                        nc.scalar.copy(out=out_ap, in_=in_ap)
                    else:
                        nc.vector.tensor_copy(out=out_ap, in_=in_ap)
                    evac_flip[0] += 1

                for j in range(4):
                    for tg in range(4):
                        ps = psC.tile([128, 512], F32R, tag="ctr")
                        for tt in range(4):
                            t = tg * 4 + tt
                            nc.tensor.transpose(
                                ps[:, tt * 128:(tt + 1) * 128],
                                cache_sb[:, t, j * 128:(j + 1) * 128],
                                ident_r,
                            )
                        evac_copy(c2t[:, j, tg * 512:(tg + 1) * 512],
                                  ps.bitcast(F32))

                # ------- phase D: projections (fp8 DoubleRow) --------------
                # v first (needed early in E), then k, then q (chunk order)
                for m in range(NT):
                    ps = psD.tile([128, DC], F32, tag="v")
                    for g in range(2):
                        nc.tensor.matmul(
                            ps, c2t[:, 2 * g:2 * g + 2, m * 128:(m + 1) * 128],
                            wv8[:, 2 * g:2 * g + 2, :],
                            start=(g == 0), stop=(g == 1), perf_mode=DR,
                        )
                    nc.vector.scalar_tensor_tensor(
                        out=v_sb[:, m, :], in0=ps, scalar=1.0, in1=bv_bc,
                        op0=ALU.mult, op1=ALU.add)
                for dst, w8, b_col in ((kT, wk8, bk_col), (qT, wq8, bq_col)):
                    for c in range(4):
                        for h in range(H):
                            ps = psD.tile([128, 512], F32, tag="qk")
                            for g in range(2):
                                nc.tensor.matmul(
                                    ps, w8[:, 2 * g:2 * g + 2, h * 128:(h + 1) * 128],
                                    c2t[:, 2 * g:2 * g + 2, c * 512:(c + 1) * 512],
                                    start=(g == 0), stop=(g == 1), perf_mode=DR,
                                )
                            if evac_flip[0] % 2 == 0:
                                nc.scalar.activation(
                                    dst[:, h, c * 512:(c + 1) * 512], ps,
                                    AF.Identity, bias=b_col[:, h:h + 1])
                            else:
                                nc.vector.tensor_scalar_add(
                                    dst[:, h, c * 512:(c + 1) * 512], ps,
                                    b_col[:, h:h + 1])
                            evac_flip[0] += 1

            # ======================= phases E + F ==========================
            with (
                tc.tile_pool(name="pEs", bufs=2) as pEs,
                tc.tile_pool(name="pF", bufs=2) as pF,
                tc.tile_pool(name="psAtt", bufs=2, space="PSUM") as psAtt,
                tc.tile_pool(name="psAo", bufs=1, space="PSUM") as psAo,
                tc.tile_pool(name="psDen", bufs=1, space="PSUM") as psDen,
                tc.tile_pool(name="psF", bufs=1, space="PSUM") as psF,
            ):
                def emit_F(c):
                    # phase F for n-tiles 2c, 2c+1 (aoT chunk c is complete)
                    mean2 = pF.tile([128, 2], F32, tag="mean2")
                    var2 = pF.tile([128, 2], F32, tag="var2")
                    r_ts = []
                    for half in range(2):
                        t = 2 * c + half
                        ps_o = psF.tile([128, DC], F32, tag="o")
                        for g in range(2):
                            nc.tensor.matmul(
                                ps_o, aoT[:, 2 * g:2 * g + 2, t * 128:(t + 1) * 128],
                                wo8[:, 2 * g:2 * g + 2, :],
                                start=(g == 0), stop=False, perf_mode=DR,
                            )
                        nc.tensor.matmul(ps_o, ones_row_bf, bor_row,
                                         start=False, stop=True)
                        r_t = pF.tile([128, DC], F32, tag="r")
                        rsum = pF.tile([128, 1], F32, tag="rsum")
                        nc.vector.scalar_tensor_tensor(
                            out=r_t, in0=ps_o, scalar=1.0,
                            in1=cache_f[:, t, :],
                            op0=ALU.mult, op1=ALU.add, accum_out=rsum,
                        )
                        nc.vector.tensor_scalar_mul(
                            mean2[:, half:half + 1], rsum, 1.0 / DC)
                        scratch = pF.tile([128, DC], F32, tag="scratch")
                        nc.vector.scalar_tensor_tensor(
                            out=scratch, in0=r_t,
                            scalar=mean2[:, half:half + 1], in1=r_t,
                            op0=ALU.subtract, op1=ALU.mult,
                            accum_out=var2[:, half:half + 1],
                        )
                        r_ts.append(r_t)
                    # rstd = 1/sqrt(var/DC + 1e-5), quake + 2 Newton (DVE only)
                    nc.vector.tensor_scalar(var2, var2, 1.0 / DC, 1e-5,
                                            ALU.mult, ALU.add)
                    vh = pF.tile([128, 2], F32, tag="vh")
                    nc.vector.tensor_scalar_mul(vh, var2, -0.5)
                    qi = pF.tile([128, 2], I32, tag="qi")
                    nc.vector.tensor_scalar(qi, var2.bitcast(I32), 1, -1,
                                            ALU.logical_shift_right, ALU.bitwise_xor)
                    rstd = pF.tile([128, 2], F32, tag="rstd")
                    nc.vector.tensor_scalar_add(rstd.bitcast(I32), qi, QMAGIC)
                    yy = pF.tile([128, 2], F32, tag="yy")
                    for _ in range(2):
                        nc.vector.tensor_tensor(yy, rstd, rstd, ALU.mult)
                        nc.vector.tensor_tensor(yy, yy, vh, ALU.mult)
                        nc.vector.tensor_scalar_add(yy, yy, 1.5)
                        nc.vector.tensor_tensor(rstd, rstd, yy, ALU.mult)
                    for half in range(2):
                        t = 2 * c + half
                        t1 = pF.tile([128, DC], F32, tag="t1")
                        nc.vector.tensor_scalar(
                            t1, r_ts[half], mean2[:, half:half + 1],
                            rstd[:, half:half + 1], ALU.subtract, ALU.mult)
                        t2 = pF.tile([128, DC], F32, tag="t2")
                        nc.vector.scalar_tensor_tensor(
                            out=t2, in0=t1, scalar=1.0, in1=lng_bc,
                            op0=ALU.mult, op1=ALU.mult,
                        )
                        o_sb = pF.tile([128, DC], F32, tag="osb")
                        nc.vector.scalar_tensor_tensor(
                            out=o_sb, in0=t2, scalar=1.0, in1=lnb_bc,
                            op0=ALU.mult, op1=ALU.add,
                        )
                        nc.sync.dma_start(out=out3[:, t, :], in_=o_sb)

                for c in range(NCH):
                    # --- QK segment: 16 tiles, exp streams into pT ---------
                    for m in range(NT):
                        ps_a = psAtt.tile([128, H * CL], F32, tag="att")
                        for h in range(H):
                            nc.tensor.matmul(
                                ps_a[:, h * CL:(h + 1) * CL],
                                kT[:, h, m * 128:(m + 1) * 128],
                                qT[:, h, c * CL:(c + 1) * CL],
                                start=True, stop=True,
                            )
                        nc.scalar.activation(
                            pT[:, :, m, :],
                            ps_a.rearrange("p (h n) -> p h n", h=H),
                            AF.Exp, scale=ATT_SCALE)

                    # --- o-projection + layernorm of the previous chunk ---
                    if c > 0:
                        emit_F(c - 1)

                    # --- PV (fp8 DoubleRow) + denominators -----------------
                    ps_aoA = psAo.tile([128, 512], F32, name=f"aoA{c}", tag="aoA")
                    ps_aoB = psAo.tile([128, 512], F32, name=f"aoB{c}", tag="aoB")
                    ps_den = psDen.tile([128, CL], F32, tag="den")
                    for j in range(NT // 2):
                        for h in range(H):
                            ps_ao = ps_aoA if h < 2 else ps_aoB
                            nc.tensor.matmul(
                                ps_ao[:, (h % 2) * CL:(h % 2 + 1) * CL],
                                v_sb[:, 2 * j:2 * j + 2, h * 128:(h + 1) * 128],
                                pT[:, h, 2 * j:2 * j + 2, :],
                                start=(j == 0 and h % 2 == 0),
                                stop=(j == NT // 2 - 1 and h % 2 == 1),
                                perf_mode=DR,
                            )
                    for m in range(NT):
                        for h in range(H):
                            nc.tensor.matmul(
                                ps_den[32 * h:32 * h + 1, :],
                                ones8,
                                pT[:, h, m, :],
                                start=(m == 0), stop=(m == NT - 1),
                                tile_position=(0, 32 * h),
                            )

                    # denominators -> 1/den, broadcast, fused normalize evac
                    den_sb = pEs.tile([128, CL], F32, tag="densb")
                    nc.vector.memset(den_sb, 1.0)
                    for h in range(H):
                        nc.vector.tensor_copy(
                            out=den_sb[32 * h:32 * h + 1, :],
                            in_=ps_den[32 * h:32 * h + 1, :],
                        )
                    rden_sb = pEs.tile([128, CL], F32, tag="rdensb")
                    nc.vector.reciprocal(rden_sb, den_sb)
                    rden = pEs.tile([1, H * CL], F32, tag="rden")
                    for h in range(H):
                        nc.sync.dma_start(
                            out=rden[:, h * CL:(h + 1) * CL],
                            in_=rden_sb[32 * h:32 * h + 1, :],
                        )
                    bc_sb = pEs.tile([128, H * CL], F32, tag="bcsb")
                    nc.gpsimd.partition_broadcast(bc_sb, rden)
                    for h in range(H):
                        ps_ao = ps_aoA if h < 2 else ps_aoB
                        nc.vector.scalar_tensor_tensor(
                            out=aoT[:, h, c * CL:(c + 1) * CL],
                            in0=ps_ao[:, (h % 2) * CL:(h % 2 + 1) * CL],
                            scalar=1.0, in1=bc_sb[:, h * CL:(h + 1) * CL],
                            op0=ALU.mult, op1=ALU.mult,
                        )

                emit_F(NCH - 1)

    nc.compile()
    return nc


_NC_CACHE = {}


def _get_nc():
    if "nc" not in _NC_CACHE:
        _NC_CACHE["nc"] = _build()
    return _NC_CACHE["nc"]


def _in_maps(inputs):
    per_batch = {"y", "cache", "gumbel_u"}
    maps = []
    for b in range(B):
        m = {}
        for name in _INPUT_SPECS:
            arr = np.ascontiguousarray(np.asarray(inputs[name], dtype=np.float32))
            m[name] = arr[b] if name in per_batch else arr
        maps.append(m)
    return maps


def _execute(inputs, trace=False):
    nc = _get_nc()
    res = run_bass_kernel_spmd(nc, _in_maps(inputs), list(range(B)), trace=trace)
    out = np.stack([res.results[b]["out"] for b in range(B)]).astype(np.float32)
    return out, res


def kernel(**inputs) -> np.ndarray:
    out, _ = _execute(inputs)
    return out


# revision 35
# speedup vs baseline: 1.3957x; 1.0068x over previous
"""DLSMN scatter-memory + cache self-attention kernel for Trainium2.

Data-parallel over batch: batch b runs on NeuronCore b (8 cores), no
collectives.  Inside one core (one batch):

  phase A: per 128-token tile of y: PE-transpose y (f32r transposes) ->
           yT (bf16), bf16 matmuls (FWL weight loads) for write_vals /
           (logits,gate), gumbel-softmax routing (all Ln batched first ->
           2 ACT table loads total), weighted-scatter matmuls into 2 PSUM
           banks + a shared mass bank (3 banks total).
  phase B: slot update  upd = (1-g)*DECAY*old + g*updates/(mass+eps).
  phase C: PE-transpose cache2 -> cache2T (fp8e4).
  phase D: q/k/v projections with fp8 DoubleRow matmuls, bias folded
           into the PSUM evacuations: qT/kT bf16, v fp8e4.
  phase E: attention transposed, QK^T bf16.  Chunk structure: all 16 QK
           tiles first (exp on ScalarE streams into a persistent fp8 pT
           buffer), then the o-projection of the PREVIOUS chunk, then PV
           (fp8 DoubleRow) + denominator matmuls.  The previous chunk's
           softmax-normalization tail overlaps this chunk's QK segment.
  phase F: pipelined one chunk behind phase E: o-projection in fp8
           DoubleRow, residual + layernorm with a DVE-only Quake rsqrt,
           output DMA per n-tile.
"""

import numpy as np

import concourse.bacc as bacc
import concourse.mybir as mybir
import concourse.tile as tile
from concourse.bass_utils import run_bass_kernel_spmd
from concourse.masks import make_identity

F32 = mybir.dt.float32
F32R = mybir.dt.float32r
F16 = mybir.dt.float16
BF16 = mybir.dt.bfloat16
FP8 = mybir.dt.float8e4
I32 = mybir.dt.int32
AF = mybir.ActivationFunctionType
ALU = mybir.AluOpType
DR = mybir.MatmulPerfMode.DoubleRow

B = 8
S = 2048
D = 1024
DC = 512
K = 256
L = 8
H = 4
HD = 128
N = L * K
LAYER_IDX = 3
DECAY = 0.9
EPS = 1e-6
ST = S // 128   # 16 token tiles
NT = N // 128   # 16 slot tiles
DCH = D // 128  # 8 d_model chunks
CL = 256        # attention n-chunk length
NCH = N // CL   # 8 attention chunks
ATT_SCALE = float(1.0 / np.sqrt(np.float32(HD)))
QMAGIC = 0x5F3759DF + 1  # quake rsqrt magic (+1 for the xor-negate trick)

_INPUT_SPECS = {
    "y": (S, D), "cache": (N, DC), "gumbel_u": (S, K),
    "W_gate": (D, 1), "b_gate": (1,), "W_slot": (D, K), "b_slot": (K,),
    "gamma": (1,), "W_write": (D, DC), "b_write": (DC,),
    "Wq": (DC, DC), "bq": (DC,), "Wk": (DC, DC), "bk": (DC,),
    "Wv": (DC, DC), "bv": (DC,), "Wo": (DC, DC), "bo": (DC,),
    "ln_g": (DC,), "ln_b": (DC,),
}


def _build():
    nc = bacc.Bacc("TRN2", target_bir_lowering=False, debug=False, num_devices=B)

    a = {
        name: nc.dram_tensor(name, list(shape), F32, kind="ExternalInput").ap()
        for name, shape in _INPUT_SPECS.items()
    }
    out_dram = nc.dram_tensor("out", [N, DC], F32, kind="ExternalOutput").ap()

    y3 = a["y"].rearrange("(t p) d -> p t d", p=128)
    gum3 = a["gumbel_u"].rearrange("(t p) k -> p t k", p=128)
    cache3 = a["cache"].rearrange("(t p) d -> p t d", p=128)
    out3 = out_dram.rearrange("(t p) d -> p t d", p=128)

    with tile.TileContext(nc) as tc:
        with (
            tc.tile_pool(name="const", bufs=1) as const,
            tc.tile_pool(name="cachep", bufs=1) as cachep,
            tc.tile_pool(name="attn", bufs=1) as attn,
        ):
            # ---------------- constants ------------------------------------
            ident = const.tile([128, 128], F32)
            make_identity(nc, ident)
            ident_bf = const.tile([128, 128], BF16)
            nc.vector.tensor_copy(out=ident_bf, in_=ident)
            ones_row_bf = const.tile([1, 128], BF16)
            nc.vector.memset(ones_row_bf, 1.0)
            ones_col2_bf = const.tile([128, 2], BF16)
            nc.vector.memset(ones_col2_bf, 1.0)
            ones8 = const.tile([128, 1], FP8)
            nc.vector.memset(ones8, 1.0)
            eps8_t = const.tile([128, 1], F32)
            nc.vector.memset(eps8_t, 1e-8)
            gamma_t = const.tile([128, 1], F32)
            nc.sync.dma_start(out=gamma_t, in_=a["gamma"].unsqueeze(0).to_broadcast([128, 1]))
            lng_bc = const.tile([128, DC], F32)
            nc.gpsimd.dma_start(out=lng_bc, in_=a["ln_g"].unsqueeze(0).to_broadcast([128, DC]))
            lnb_bc = const.tile([128, DC], F32)
            nc.gpsimd.dma_start(out=lnb_bc, in_=a["ln_b"].unsqueeze(0).to_broadcast([128, DC]))
            bwr_bc = const.tile([128, DC], F32)
            nc.gpsimd.dma_start(out=bwr_bc, in_=a["b_write"].unsqueeze(0).to_broadcast([128, DC]))
            bv_bc = const.tile([128, DC], F32)
            nc.gpsimd.dma_start(out=bv_bc, in_=a["bv"].unsqueeze(0).to_broadcast([128, DC]))
            # per-partition bias columns for q/k (out partition = dc within head)
            bq_col = const.tile([128, H], F32)
            nc.gpsimd.dma_start(out=bq_col, in_=a["bq"].rearrange("(h p) -> p h", p=128))
            bk_col = const.tile([128, H], F32)
            nc.gpsimd.dma_start(out=bk_col, in_=a["bk"].rearrange("(h p) -> p h", p=128))
            bsg_row = const.tile([1, K + 2], BF16)
            nc.gpsimd.dma_start(out=bsg_row[:, 0:K], in_=a["b_slot"].unsqueeze(0))
            nc.gpsimd.dma_start(out=bsg_row[:, K:K + 1], in_=a["b_gate"].unsqueeze(0))
            nc.gpsimd.dma_start(out=bsg_row[:, K + 1:K + 2], in_=a["b_gate"].unsqueeze(0))
            bor_row = const.tile([1, DC], BF16)
            nc.gpsimd.dma_start(out=bor_row, in_=a["bo"].unsqueeze(0))

            cache_sb = cachep.tile([128, NT, DC], BF16)

            # ---------------- persistent attention tiles -------------------
            c2t = attn.tile([128, 4, N], FP8)
            qT = attn.tile([128, H, N], BF16)
            kT = attn.tile([128, H, N], BF16)
            v_sb = attn.tile([128, NT, DC], FP8)
            wq8 = attn.tile([128, 4, DC], FP8)
            wk8 = attn.tile([128, 4, DC], FP8)
            wv8 = attn.tile([128, 4, DC], FP8)
            wo8 = attn.tile([128, 4, DC], FP8)
            aoT = attn.tile([128, H, N], FP8)
            pT = attn.tile([128, H, NT, CL], FP8)

            # ======================= phase A + B ===========================
            with (
                tc.tile_pool(name="wA", bufs=1) as wA,
                tc.tile_pool(name="pA", bufs=2) as pA,
                tc.tile_pool(name="pAs", bufs=3) as pAs,
                tc.tile_pool(name="psT", bufs=2, space="PSUM") as psT,
                tc.tile_pool(name="psWV", bufs=2, space="PSUM") as psWV,
                tc.tile_pool(name="psLG", bufs=1, space="PSUM") as psLG,
                tc.tile_pool(name="psU", bufs=1, space="PSUM") as psU,
            ):
                # weights: casting DMAs on the gpsimd queue
                wwr = wA.tile([128, DCH, DC], BF16)
                wsg = wA.tile([128, DCH, K + 2], BF16)
                wwr3 = a["W_write"].rearrange("(c p) d -> p c d", p=128)
                wsl3 = a["W_slot"].rearrange("(c p) k -> p c k", p=128)
                nc.gpsimd.dma_start(out=wwr, in_=wwr3)
                nc.gpsimd.dma_start(out=wsg[:, :, 0:K], in_=wsl3)
                nc.gpsimd.dma_start(out=wsg[:, :, K:K + 1], in_=a["W_gate"].rearrange("(c p) o -> p c o", p=128))
                nc.gpsimd.dma_start(out=wsg[:, :, K + 1:K + 2], in_=a["W_gate"].rearrange("(c p) o -> p c o", p=128))
                # cache for phases B/C/F (casting DMA fp32 -> bf16)
                nc.gpsimd.dma_start(out=cache_sb, in_=cache3)
                # attention weights (fp8/bf16 casting DMAs)
                wq3 = a["Wq"].rearrange("(c p) d -> p c d", p=128)
                wk3 = a["Wk"].rearrange("(c p) d -> p c d", p=128)
                wv3 = a["Wv"].rearrange("(c p) d -> p c d", p=128)
                wo3 = a["Wo"].rearrange("(c p) d -> p c d", p=128)
                nc.gpsimd.dma_start(out=wq8, in_=wq3)
                nc.gpsimd.dma_start(out=wk8, in_=wk3)
                nc.gpsimd.dma_start(out=wv8, in_=wv3)
                nc.gpsimd.dma_start(out=wo8, in_=wo3)

                # prologue: first two y tiles DMA'd + cast (DVE) before the
                # Ln prepass so the PE transposes start immediately
                y_bf_pre = {}
                for i in range(2):
                    y_t = pA.tile([128, D], F32, tag="y", name=f"ypre{i}")
                    nc.sync.dma_start(out=y_t, in_=y3[:, i, :])
                    y_bf = pA.tile([128, D], BF16, tag="ybf", name=f"ybfpre{i}")
                    nc.vector.tensor_copy(out=y_bf, in_=y_t)
                    y_bf_pre[i] = y_bf

                # gumbel Ln prepass: all Ln ops batched (one ACT table set)
                lnz_all = wA.tile([128, ST, K], F16)
                for g in range(4):
                    gum = pA.tile([128, 4, K], F32, tag="gum")
                    nc.sync.dma_start(out=gum, in_=gum3[:, 4 * g:4 * g + 4, :])
                    lnu = pA.tile([128, 4, K], F32, tag="lnu")
                    nc.scalar.activation(lnu, gum, AF.Ln, bias=eps8_t)
                    nc.scalar.activation(lnz_all[:, 4 * g:4 * g + 4, :], lnu, AF.Ln,
                                         bias=eps8_t, scale=-1.0)

                # persistent scatter accumulators:
                #   updates: one bank per kc (512 fp32)
                #   mass: single shared bank [128, 4] (cols 2kc:2kc+2)
                ps_upd = [psU.tile([128, DC], F32, name=f"upd{kc}", tag=f"upd{kc}")
                          for kc in range(2)]
                ps_mass = psU.tile([128, 4], F32, name="mass", tag="mass")

                prev = [None]

                def flush_scatter():
                    if prev[0] is None:
                        return
                    j, w_j, wv_j = prev[0]
                    for kc in range(2):
                        lhs = w_j[:, kc * 128:(kc + 1) * 128]
                        nc.tensor.matmul(ps_upd[kc], lhs, wv_j,
                                         start=(j == 0), stop=(j == ST - 1))
                        nc.tensor.matmul(ps_mass[:, 2 * kc:2 * kc + 2], lhs,
                                         ones_col2_bf,
                                         start=(j == 0 and kc == 0),
                                         stop=(j == ST - 1 and kc == 1),
                                         skip_group_check=True)
                    prev[0] = None

                for i in range(ST):
                    if i in y_bf_pre:
                        y_bf = y_bf_pre[i]
                    else:
                        y_t = pA.tile([128, D], F32, tag="y")
                        nc.sync.dma_start(out=y_t, in_=y3[:, i, :])
                        y_bf = pA.tile([128, D], BF16, tag="ybf")
                        nc.scalar.copy(out=y_bf, in_=y_t)

                    # transpose y tile -> yT (bf16 transposes, cheap LDW)
                    yT = pA.tile([128, D], BF16, tag="yT")
                    for g in range(2):
                        tr = psT.tile([128, 512], BF16, tag="tr")
                        for cc in range(4):
                            c = 4 * g + cc
                            nc.tensor.transpose(
                                tr[:, cc * 128:(cc + 1) * 128],
                                y_bf[:, c * 128:(c + 1) * 128],
                                ident_bf,
                            )
                        nc.vector.tensor_copy(out=yT[:, g * 512:(g + 1) * 512],
                                              in_=tr)

                    # write_vals / (logits, gate) matmuls
                    ps_wv = psWV.tile([128, DC], F32, tag="wv")
                    for c in range(DCH):
                        nc.tensor.matmul(
                            ps_wv, yT[:, c * 128:(c + 1) * 128], wwr[:, c, :],
                            start=(c == 0), stop=(c == DCH - 1),
                        )
                    ps_lg = psLG.tile([128, K + 2], F32, tag="lg")
                    for c in range(DCH):
                        nc.tensor.matmul(
                            ps_lg, yT[:, c * 128:(c + 1) * 128], wsg[:, c, :],
                            start=(c == 0), stop=False,
                        )
                    nc.tensor.matmul(ps_lg, ones_row_bf, bsg_row,
                                     start=False, stop=True)

                    # scatter matmuls for the previous tile (keeps PE dense
                    # while this tile's DVE/ACT chain runs)
                    flush_scatter()

                    # t = gamma*logits - lnz
                    t_sb = pAs.tile([128, K], F32, tag="tsb")
                    nc.vector.scalar_tensor_tensor(
                        out=t_sb, in0=ps_lg[:, 0:K], scalar=gamma_t,
                        in1=lnz_all[:, i, :], op0=ALU.mult, op1=ALU.subtract,
                    )
                    # scores = sigmoid(gate)
                    sc_e = pAs.tile([128, 1], F32, tag="sce")
                    nc.scalar.activation(sc_e, ps_lg[:, K:K + 1], AF.Exp, scale=-1.0)
                    sc1 = pAs.tile([128, 1], F32, tag="sc1")
                    nc.vector.tensor_scalar_add(sc1, sc_e, 1.0)
                    scores = pAs.tile([128, 1], F32, tag="scores")
                    nc.vector.reciprocal(scores, sc1)
                    # p_unnorm = exp(t) with fused row-sum
                    p_un = pAs.tile([128, K], F32, tag="pun")
                    rs = pAs.tile([128, 1], F32, tag="rs")
                    nc.scalar.activation(p_un, t_sb, AF.Exp, accum_out=rs)
                    rrs = pAs.tile([128, 1], F32, tag="rrs")
                    nc.vector.reciprocal(rrs, rs)
                    s2 = pAs.tile([128, 1], F32, tag="s2")
                    nc.vector.tensor_tensor(s2, scores, rrs, ALU.mult)
                    w_sb = pAs.tile([128, K], BF16, tag="wsb")
                    nc.vector.tensor_scalar_mul(w_sb, p_un, s2)
                    # wv_sb = write_vals + b_write
                    wv_sb = pAs.tile([128, DC], BF16, tag="wvsb")
                    nc.vector.scalar_tensor_tensor(
                        out=wv_sb, in0=ps_wv, scalar=1.0,
                        in1=bwr_bc, op0=ALU.mult, op1=ALU.add,
                    )
                    prev[0] = (i, w_sb, wv_sb)

                flush_scatter()

                # ------- phase B: slot update, overwrite cache rows -------
                base_t = LAYER_IDX * K // 128  # n-tile 6
                for kc in range(2):
                    mass = pAs.tile([128, 1], F32, tag="mass")
                    nc.vector.tensor_copy(out=mass, in_=ps_mass[:, 2 * kc:2 * kc + 1])
                    m1 = pAs.tile([128, 1], F32, tag="m1")
                    nc.vector.tensor_scalar_add(m1, mass, EPS)
                    rm = pAs.tile([128, 1], F32, tag="rm")
                    nc.vector.reciprocal(rm, m1)
                    m2 = pAs.tile([128, 1], F32, tag="m2")
                    nc.vector.tensor_scalar_add(m2, mass, 1.0)
                    rg = pAs.tile([128, 1], F32, tag="rg")
                    nc.vector.reciprocal(rg, m2)
                    g_t = pAs.tile([128, 1], F32, tag="gt")
                    nc.vector.tensor_tensor(g_t, mass, rg, ALU.mult)
                    co = pAs.tile([128, 1], F32, tag="co")
                    nc.vector.tensor_scalar(co, g_t, -DECAY, DECAY, ALU.mult, ALU.add)
                    cn = pAs.tile([128, 1], F32, tag="cn")
                    nc.vector.tensor_tensor(cn, g_t, rm, ALU.mult)

                    told = pAs.tile([128, DC], F32, tag="told")
                    nc.vector.tensor_scalar_mul(told, cache_sb[:, base_t + kc, :], co)
                    nc.vector.scalar_tensor_tensor(
                        out=cache_sb[:, base_t + kc, :],
                        in0=ps_upd[kc], scalar=cn, in1=told,
                        op0=ALU.mult, op1=ALU.add,
                    )

            # ======================= phases C + D ==========================
            with (
                tc.tile_pool(name="psC", bufs=2, space="PSUM") as psC,
                tc.tile_pool(name="psD", bufs=3, space="PSUM") as psD,
            ):
                # ------- phase C: cache2 -> cache2T (fp8) ------------------
                evac_flip = [0]

                def evac_copy(out_ap, in_ap):
                    if evac_flip[0] % 2 == 0:
                        nc.scalar.copy(out=out_ap, in_=in_ap)
                    else:
                        nc.vector.tensor_copy(out=out_ap, in_=in_ap)
                    evac_flip[0] += 1

                for j in range(4):
                    for tg in range(4):
                        ps = psC.tile([128, 512], BF16, tag="ctr")
                        for tt in range(4):
                            t = tg * 4 + tt
                            nc.tensor.transpose(
                                ps[:, tt * 128:(tt + 1) * 128],
                                cache_sb[:, t, j * 128:(j + 1) * 128],
                                ident_bf,
                            )
                        evac_copy(c2t[:, j, tg * 512:(tg + 1) * 512], ps)

                # ------- phase D: projections (fp8 DoubleRow) --------------
                # v first (needed early in E), then k, then q (chunk order)
                for m in range(NT):
                    ps = psD.tile([128, DC], F32, tag="v")
                    for g in range(2):
                        nc.tensor.matmul(
                            ps, c2t[:, 2 * g:2 * g + 2, m * 128:(m + 1) * 128],
                            wv8[:, 2 * g:2 * g + 2, :],
                            start=(g == 0), stop=(g == 1), perf_mode=DR,
                        )
                    nc.vector.scalar_tensor_tensor(
                        out=v_sb[:, m, :], in0=ps, scalar=1.0, in1=bv_bc,
                        op0=ALU.mult, op1=ALU.add)
                for dst, w8, b_col in ((kT, wk8, bk_col), (qT, wq8, bq_col)):
                    for c in range(4):
                        for h in range(H):
                            ps = psD.tile([128, 512], F32, tag="qk")
                            for g in range(2):
                                nc.tensor.matmul(
                                    ps, w8[:, 2 * g:2 * g + 2, h * 128:(h + 1) * 128],
                                    c2t[:, 2 * g:2 * g + 2, c * 512:(c + 1) * 512],
                                    start=(g == 0), stop=(g == 1), perf_mode=DR,
                                )
                            if evac_flip[0] % 2 == 0:
                                nc.scalar.activation(
                                    dst[:, h, c * 512:(c + 1) * 512], ps,
                                    AF.Identity, bias=b_col[:, h:h + 1])
                            else:
                                nc.vector.tensor_scalar_add(
                                    dst[:, h, c * 512:(c + 1) * 512], ps,
                                    b_col[:, h:h + 1])
                            evac_flip[0] += 1

            # ======================= phases E + F ==========================
            with (
                tc.tile_pool(name="pEs", bufs=2) as pEs,
                tc.tile_pool(name="pF", bufs=2) as pF,
                tc.tile_pool(name="psAtt", bufs=2, space="PSUM") as psAtt,
                tc.tile_pool(name="psAo", bufs=1, space="PSUM") as psAo,
                tc.tile_pool(name="psDen", bufs=1, space="PSUM") as psDen,
                tc.tile_pool(name="psF", bufs=1, space="PSUM") as psF,
            ):
                def emit_F(c):
                    # phase F for n-tiles 2c, 2c+1 (aoT chunk c is complete)
                    mean2 = pF.tile([128, 2], F32, tag="mean2")
                    var2 = pF.tile([128, 2], F32, tag="var2")
                    r_ts = []
                    for half in range(2):
                        t = 2 * c + half
                        ps_o = psF.tile([128, DC], F32, tag="o")
                        for g in range(2):
                            nc.tensor.matmul(
                                ps_o, aoT[:, 2 * g:2 * g + 2, t * 128:(t + 1) * 128],
                                wo8[:, 2 * g:2 * g + 2, :],
                                start=(g == 0), stop=False, perf_mode=DR,
                            )
                        nc.tensor.matmul(ps_o, ones_row_bf, bor_row,
                                         start=False, stop=True)
                        r_t = pF.tile([128, DC], F32, tag="r")
                        rsum = pF.tile([128, 1], F32, tag="rsum")
                        nc.vector.scalar_tensor_tensor(
                            out=r_t, in0=ps_o, scalar=1.0,
                            in1=cache_sb[:, t, :],
                            op0=ALU.mult, op1=ALU.add, accum_out=rsum,
                        )
                        nc.vector.tensor_scalar_mul(
                            mean2[:, half:half + 1], rsum, 1.0 / DC)
                        scratch = pF.tile([128, DC], F32, tag="scratch")
                        nc.vector.scalar_tensor_tensor(
                            out=scratch, in0=r_t,
                            scalar=mean2[:, half:half + 1], in1=r_t,
                            op0=ALU.subtract, op1=ALU.mult,
                            accum_out=var2[:, half:half + 1],
                        )
                        r_ts.append(r_t)
                    # rstd = 1/sqrt(var/DC + 1e-5), quake + 2 Newton (DVE only)
                    nc.vector.tensor_scalar(var2, var2, 1.0 / DC, 1e-5,
                                            ALU.mult, ALU.add)
                    vh = pF.tile([128, 2], F32, tag="vh")
                    nc.vector.tensor_scalar_mul(vh, var2, -0.5)
                    qi = pF.tile([128, 2], I32, tag="qi")
                    nc.vector.tensor_scalar(qi, var2.bitcast(I32), 1, -1,
                                            ALU.logical_shift_right, ALU.bitwise_xor)
                    rstd = pF.tile([128, 2], F32, tag="rstd")
                    nc.vector.tensor_scalar_add(rstd.bitcast(I32), qi, QMAGIC)
                    yy = pF.tile([128, 2], F32, tag="yy")
                    for _ in range(2):
                        nc.vector.tensor_tensor(yy, rstd, rstd, ALU.mult)
                        nc.vector.tensor_tensor(yy, yy, vh, ALU.mult)
                        nc.vector.tensor_scalar_add(yy, yy, 1.5)
                        nc.vector.tensor_tensor(rstd, rstd, yy, ALU.mult)
                    for half in range(2):
                        t = 2 * c + half
                        t1 = pF.tile([128, DC], F32, tag="t1")
                        nc.vector.tensor_scalar(
                            t1, r_ts[half], mean2[:, half:half + 1],
                            rstd[:, half:half + 1], ALU.subtract, ALU.mult)
                        t2 = pF.tile([128, DC], F32, tag="t2")
                        nc.vector.scalar_tensor_tensor(
                            out=t2, in0=t1, scalar=1.0, in1=lng_bc,
                            op0=ALU.mult, op1=ALU.mult,
                        )
                        o_sb = pF.tile([128, DC], F32, tag="osb")
                        nc.vector.scalar_tensor_tensor(
                            out=o_sb, in0=t2, scalar=1.0, in1=lnb_bc,
                            op0=ALU.mult, op1=ALU.add,
                        )
                        nc.sync.dma_start(out=out3[:, t, :], in_=o_sb)

                for c in range(NCH):
                    # QK/exp stream with PV (fp8 DoubleRow) + denominator
                    # matmuls interleaved once the previous chunk's ao banks
                    # have drained (pairs 0..6 after QK m=9..15, pair 7 last)
                    ps_aoA = psAo.tile([128, 512], F32, name=f"aoA{c}", tag="aoA")
                    ps_aoB = psAo.tile([128, 512], F32, name=f"aoB{c}", tag="aoB")
                    ps_den = psDen.tile([128, CL], F32, tag="den")

                    def emit_pair(j):
                        for h in range(H):
                            ps_ao = ps_aoA if h < 2 else ps_aoB
                            nc.tensor.matmul(
                                ps_ao[:, (h % 2) * CL:(h % 2 + 1) * CL],
                                v_sb[:, 2 * j:2 * j + 2, h * 128:(h + 1) * 128],
                                pT[:, h, 2 * j:2 * j + 2, :],
                                start=(j == 0 and h % 2 == 0),
                                stop=(j == NT // 2 - 1 and h % 2 == 1),
                                perf_mode=DR,
                            )
                        for m2 in (2 * j, 2 * j + 1):
                            for h in range(H):
                                nc.tensor.matmul(
                                    ps_den[32 * h:32 * h + 1, :],
                                    ones8,
                                    pT[:, h, m2, :],
                                    start=(m2 == 0), stop=(m2 == NT - 1),
                                    tile_position=(0, 32 * h),
                                )

                    for m in range(NT):
                        ps_a = psAtt.tile([128, H * CL], F32, tag="att")
                        for h in range(H):
                            nc.tensor.matmul(
                                ps_a[:, h * CL:(h + 1) * CL],
                                kT[:, h, m * 128:(m + 1) * 128],
                                qT[:, h, c * CL:(c + 1) * CL],
                                start=True, stop=True,
                            )
                        nc.scalar.activation(
                            pT[:, :, m, :],
                            ps_a.rearrange("p (h n) -> p h n", h=H),
                            AF.Exp, scale=ATT_SCALE)
                        if m == 10 and c > 0:
                            emit_F(c - 1)
                        if m >= 9:
                            emit_pair(m - 9)
                    emit_pair(NT // 2 - 1)

                    # denominators -> 1/den, broadcast, fused normalize evac
                    den_sb = pEs.tile([128, CL], F32, tag="densb")
                    nc.vector.memset(den_sb, 1.0)
                    for h in range(H):
                        nc.vector.tensor_copy(
                            out=den_sb[32 * h:32 * h + 1, :],
                            in_=ps_den[32 * h:32 * h + 1, :],
                        )
                    rden_sb = pEs.tile([128, CL], F32, tag="rdensb")
                    nc.vector.reciprocal(rden_sb, den_sb)
                    rden = pEs.tile([1, H * CL], F32, tag="rden")
                    for h in range(H):
                        nc.sync.dma_start(
                            out=rden[:, h * CL:(h + 1) * CL],
                            in_=rden_sb[32 * h:32 * h + 1, :],
                        )
                    bc_sb = pEs.tile([128, H * CL], F32, tag="bcsb")
                    nc.gpsimd.partition_broadcast(bc_sb, rden)
                    for h in range(H):
                        ps_ao = ps_aoA if h < 2 else ps_aoB
                        nc.vector.scalar_tensor_tensor(
                            out=aoT[:, h, c * CL:(c + 1) * CL],
                            in0=ps_ao[:, (h % 2) * CL:(h % 2 + 1) * CL],
                            scalar=1.0, in1=bc_sb[:, h * CL:(h + 1) * CL],
                            op0=ALU.mult, op1=ALU.mult,
                        )

                emit_F(NCH - 1)

    nc.compile()
    return nc


_NC_CACHE = {}


def _get_nc():
    if "nc" not in _NC_CACHE:
        _NC_CACHE["nc"] = _build()
    return _NC_CACHE["nc"]


def _in_maps(inputs):
    per_batch = {"y", "cache", "gumbel_u"}
    maps = []
    for b in range(B):
        m = {}
        for name in _INPUT_SPECS:
            arr = np.ascontiguousarray(np.asarray(inputs[name], dtype=np.float32))
            m[name] = arr[b] if name in per_batch else arr
        maps.append(m)
    return maps


def _execute(inputs, trace=False):
    nc = _get_nc()
    res = run_bass_kernel_spmd(nc, _in_maps(inputs), list(range(B)), trace=trace)
    out = np.stack([res.results[b]["out"] for b in range(B)]).astype(np.float32)
    return out, res


def kernel(**inputs) -> np.ndarray:
    out, _ = _execute(inputs)
    return out
